# revision 2
# baseline (speedup 1.0000x reference)
# Trainium2 Bass kernel for nn_Encoder_SelfAttention (sparse_attention), v2.
#
# Same contract as the baseline: kernel(**inputs) takes FULL unsharded inputs,
# shards batch across 8 cores, returns FULL (8,512,512) f32 output.
#
# v2 redesign (vs baseline at 144.5us):
# - Scores per (head, kt-block) built by ONE fused fp16 matmul with an
#   augmented K=112 contraction: rows 0..63 = q/k head rows (plain qk^T),
#   rows 64..87 = rank-24 SVD of -c*g^2, rows 88..111 = rank-12 SVD of g
#   paired with per-head 2c*dq / 2c*dk scaled basis rows. The tiny
#   -c*(dq+dk)^2 rank-1 terms (max 4e-3 in score units) are dropped.
#   PE matmul cost depends only on output columns, so folding all bias
#   terms into the contraction removes 4 of 5 score passes.
# - err_order: z = sign*(oq+ok) built by DVE/Pool stt into fp16; softplus as
#   a single ACT op (AF.Softplus, validated on hw; Exp+Ln fallback);
#   softplus SUBTRACTION done on PE via a -I fp16 matmul accumulated into
#   the scores PSUM group, so the final exp reads PSUM directly.
# - softmax denominator via ones-row in vaug (as baseline); reciprocal on
#   DVE (nc.vector.reciprocal), broadcast by a tiny PE matmul, and the
#   normalizing multiply on Pool (gpsimd) to keep DVE/ACT free.
# - FFT filter: fp16 DFT bases, Nyquist frequency dropped (validated
#   ~1e-3 end-to-end), filter products on DVE in fp16 (2x mode).
# - All big operands fp16 (weights, x for matmuls, sign matrix, bases):
#   halves DMA bytes; DMAs merged into one descriptor-batch per symbol.
# - z/softplus for ALL heads precomputed concurrently with projections so
#   the per-head PE stream (fused mm, -sp mm, ctx mm) never stalls on ACT.
import sys

sys.path.insert(0, "/opt/trn_rl_repo")

import numpy as np
from contextlib import ExitStack

import concourse.bass as bass
import concourse.tile as tile
from concourse import mybir
from concourse.bass_utils import run_bass_kernel_spmd
from concourse.masks import make_identity
from concourse.vector_clock import ScopedClock, VectorClock

F32 = mybir.dt.float32
F16 = mybir.dt.float16
R = mybir.dt.float32r
AF = mybir.ActivationFunctionType
ALU = mybir.AluOpType
B, S, H, NH, D = 8, 512, 512, 8, 64
NT = 4
R2, R1 = 24, 12          # SVD ranks for g^2 and g
KQ = 64                   # q/k head rows
KA = KQ + R2 + 2 * R1     # 112 fused contraction rows (113 with mask row)
NF = 256                  # kept rfft frequencies (Nyquist dropped)
# 32-aligned sub-blocks of the fused contraction (engine partition-start rule):
#   64..75  U (lhs, stt in0)      | V*2c*dq (rhs, stt out)
#   76..95  P[0:20]               | -c*S2[0:20]
#   96..107 U*2c*dk (lhs stt out) | V (rhs, stt in0)
#   108..111 P[20:24]             | -c*S2[20:24]
RU, RP0, RS, RP1 = 64, 76, 96, 108


class _TileContext(tile.TileContext):
    # This walrus build rejects >1 sem wait on SP CTRL instructions; split
    # the tail-drain global-clock waits one-per-NOP. (Same as baseline.)
    def _drain_and_barrier(self, tick_clock, wait_clock):
        g = tick_clock.global_clock
        n = len(g)
        for i in range(n):
            if g[i] > 0:
                vec = [0] * n
                vec[i] = g[i]
                nop_inst = self.nc.sync.nop(nofuse=True)
                wait_clock.add_sem_waits(
                    nop_inst.ins, ScopedClock({None: VectorClock(vec)})
                )
        self.nc.sync.drain()
        self.nc.all_engine_barrier()
        assert self.sems is not None
        popped = self.nc._tile_sem_poison_stack.pop()
        assert popped is self._sem_poison
        self.nc.clear_and_free_semaphores(list(self.sems.allocated().values()))
        self.nc.all_engine_barrier()


def _split_excess_waits(nc):
    """Spill >cap sync-waits onto injected same-engine NOPs (walrus quirk)."""
    import bass_rust

    total = 0
    for fn in nc.m.functions:
        for blk in fn.blocks:
            out = []
            for inst in blk.instructions:
                si = inst.sync_info
                waits = list(si.on_wait) if si is not None else []
                cap = 2 if inst.__class__.__name__ == "InstEventSemaphore" else 1
                if len(waits) > cap:
                    keep, spill = waits[:cap], waits[cap:]
                    for w in spill:
                        nop = mybir.InstNoOp(
                            name=f"wsplit-{inst.name}-{total}", ins=[], outs=[])
                        nop.engine = inst.engine
                        nop.sync_info = bass_rust.SyncInfo(on_wait=[w], on_update=[])
                        out.append(nop)
                        total += 1
                    inst.sync_info = bass_rust.SyncInfo(
                        on_wait=keep, on_update=list(si.on_update))
                out.append(inst)
            blk.instructions = out
    return total


_HC = None


def _host_constants():
    """Input-independent structural constants (cached)."""
    global _HC
    if _HC is not None:
        return _HC
    idx = np.arange(S)
    g = np.log(np.abs(idx[None, :] - idx[:, None]).astype(np.float64) + 1.0)
    g2 = g ** 2
    u2, s2, vt2 = np.linalg.svd(g2)
    P2 = u2[:, :R2] * np.sqrt(s2[:R2])
    S2c = vt2[:R2].T * np.sqrt(s2[:R2])          # g2 ~= P2 @ S2c.T
    u1, s1, vt1 = np.linalg.svd(g)
    U1 = u1[:, :R1] * np.sqrt(s1[:R1])
    V1 = vt1[:R1].T * np.sqrt(s1[:R1])           # g ~= U1 @ V1.T
    # rfft/irfft ortho bases, Nyquist (freq 256) dropped
    W = np.fft.rfft(np.eye(H), norm="ortho", axis=-1)
    cret = np.ascontiguousarray(W.real[:, :NF]).astype(np.float16)   # [H, NF]
    cimt = np.ascontiguousarray(W.imag[:, :NF]).astype(np.float16)
    irA = np.fft.irfft(np.eye(257), n=H, norm="ortho", axis=-1)[:NF].astype(np.float16)
    irB = np.fft.irfft(1j * np.eye(257), n=H, norm="ortho", axis=-1)[:NF].astype(np.float16)
    ssign = np.where(idx[:, None] > idx[None, :], -1.0, 1.0).astype(np.float16)  # [k,q]
    onesel = np.zeros((NH, NH * 128), np.float32)
    for h in range(NH):
        onesel[h, h * 128:(h + 1) * 128] = 1.0
    sel16 = np.zeros((NH, NH * R1), np.float16)  # dk/dq head-row selector
    for h in range(NH):
        sel16[h, h * R1:(h + 1) * R1] = 1.0
    _HC = dict(g=g, g2=g2, P2=P2, S2c=S2c, U1=U1, V1=V1,
               cret=cret, cimt=cimt, irA=irA, irB=irB,
               ss=ssign, onesel=onesel, sel16=sel16)
    return _HC


def _build_program(c, flags):
    hc = _host_constants()
    nc = bass.Bass("TRN2", target_bir_lowering=False, debug=False)
    negc = -c["c"]
    twoc = 2.0 * c["c"]
    KF = KA + 1 if flags["use_mask"] else KA   # fused contraction depth

    def din(name, shape, dt):
        return nc.dram_tensor(name, list(shape), dt, kind="ExternalInput").ap()

    x_d = din("x", (S, H), F16)
    wq_d = din("wq", (H, H), F16)
    wk_d = din("wk", (H, H), F16)
    wv_d = din("wv", (H, H), F16)
    wblkq_d = din("wblkq", (H, 16), F16)
    wblkk_d = din("wblkk", (H, 16), F16)
    ss_d = din("ss", (S, S), F16)
    dlA_d = din("dlA", (32, S), F16)         # [U1^T ; P2^T[0:20]]
    dlB_d = din("dlB", (4, S), F16)          # P2^T[20:24]
    drA_d = din("drA", (20, S), F16)         # -c*S2c^T[0:20]
    drB_d = din("drB", (R1, S), F16)         # V1^T
    drC_d = din("drC", (4, S), F16)          # -c*S2c^T[20:24]
    cret_d = din("cret", (H, NF), F16)
    cimt_d = din("cimt", (H, NF), F16)
    irA_d = din("irA", (NF, H), F16)
    irB_d = din("irB", (NF, H), F16)
    wrt_d = din("wrt", (NF, S), F16)
    wit_d = din("wit", (NF, S), F16)
    onesel_d = din("onesel", (NH, NH * 128), R)
    sel16_d = din("sel16", (NH, NH * R1), F16)
    if flags["use_mask"]:
        m8_d = din("m8", (S,), F16)
    if flags["use_bq"]:
        bq_d = din("bq", (H,), F32)
    if flags["use_bk"]:
        bk_d = din("bk", (H,), F32)
    if flags["use_bv"]:
        bv_d = din("bv", (H,), F32)
    ln_bcast = {}
    for nm in ("lnfw", "lnfb", "lnw", "lnb"):
        if flags["use_" + nm]:
            ln_bcast[nm] = din(nm, (H,), F32)
    out_d = nc.dram_tensor("out", [S, H], F32, kind="ExternalOutput").ap()
    import os
    dbg = os.environ.get("KERNEL_DEBUG", "") == "1"
    dbg_d = {}
    if dbg:
        for nm, shape, dt in (("d_xt0", (128, S), F16), ("d_lhs", (KF, NH * S), F16),
                              ("d_rhs", (KF, NH * S), F16), ("d_sp0", (128, NT * S), F16),
                              ("d_et0", (128, NT * S), F32), ("d_cps0", (65, S), F32),
                              ("d_ctxt0", (128, S), F16), ("d_rows_oq", (8, S), F32),
                              ("d_okc0", (128, 8), F32)):
            dbg_d[nm] = nc.dram_tensor(nm, list(shape), dt, kind="ExternalOutput").ap()

    def blk_ap(d, rows, width, nblk, rep=False):
        """3D ap: HBM [rows*nblk, width] -> SBUF [rows, nblk*width].
        rep=True re-reads the same [rows,width] block nblk times."""
        return bass.AP(tensor=d.tensor, offset=0,
                       ap=[[width, rows], [0 if rep else rows * width, nblk],
                           [1, width]])

    def rep_load(engine, dst_tile_slice, d, rows, width, nblk):
        """Replicated load as nblk separate DMAs (no zero-stride free dim)."""
        ap0 = dst_tile_slice
        for b in range(nblk):
            sub = bass.AP(tensor=ap0.tensor, offset=ap0.offset + b * width,
                          ap=[list(ap0.ap[0]), [1, width]])
            engine.dma_start(sub, bass.AP(tensor=d.tensor, offset=0,
                                          ap=[[width, rows], [1, width]]))

    with _TileContext(nc) as tc:
        with ExitStack() as ctx:
            consts = ctx.enter_context(tc.tile_pool(name="consts", bufs=1))
            work = ctx.enter_context(tc.tile_pool(name="work", bufs=2))
            etp = ctx.enter_context(tc.tile_pool(name="etp", bufs=2))
            small = ctx.enter_context(tc.tile_pool(name="small", bufs=2))

            # ---- DMA loads (merged, ordered by first use) ----
            x16 = consts.tile([128, NT * S], F16, tag="x16")
            nc.sync.dma_start(x16[:, 0:2 * S],
                              bass.AP(tensor=x_d.tensor, offset=0,
                                      ap=[[S, 128], [128 * S, 2], [1, S]]))
            nc.sync.dma_start(x16[:, 2 * S:],
                              bass.AP(tensor=x_d.tensor, offset=2 * 128 * S,
                                      ap=[[S, 128], [128 * S, 2], [1, S]]))
            wq16 = consts.tile([128, NT * S], F16, tag="wq16")
            nc.scalar.dma_start(wq16[:], blk_ap(wq_d, 128, S, NT))
            wk16 = consts.tile([128, NT * S], F16, tag="wk16")
            nc.sync.dma_start(wk16[:], blk_ap(wk_d, 128, S, NT))
            wblkq16 = consts.tile([128, NT * 16], F16, tag="wblkq16")
            nc.scalar.dma_start(wblkq16[:], blk_ap(wblkq_d, 128, 16, NT))
            wblkk16 = consts.tile([128, NT * 16], F16, tag="wblkk16")
            nc.scalar.dma_start(wblkk16[:], blk_ap(wblkk_d, 128, 16, NT))
            sst = consts.tile([128, NT * S], F16, tag="sst")
            nc.sync.dma_start(sst[:], blk_ap(ss_d, 128, S, NT))
            onesel_t = consts.tile([NH, NH * 128], R, tag="onesel")
            nc.sync.dma_start(onesel_t[:], onesel_d)
            sel16_t = consts.tile([NH, NH * R1], F16, tag="sel16")
            nc.sync.dma_start(sel16_t[:], sel16_d)

            # Fused-contraction operand tiles; aug rows replicated x8 by DMA
            lhs_all = consts.tile([KF, NH * S], F16, tag="lhs_all", name="lhs_all")
            rhs_all = consts.tile([KF, NH * S], F16, tag="rhs_all", name="rhs_all")
            nc.scalar.dma_start(lhs_all[RU:RU + 32, :], blk_ap(dlA_d, 32, S, NH, rep=True))
            nc.scalar.dma_start(lhs_all[RP1:RP1 + 4, :], blk_ap(dlB_d, 4, S, NH, rep=True))
            nc.sync.dma_start(rhs_all[RP0:RP0 + 20, :], blk_ap(drA_d, 20, S, NH, rep=True))
            nc.sync.dma_start(rhs_all[RS:RS + R1, :], blk_ap(drB_d, R1, S, NH, rep=True))
            nc.sync.dma_start(rhs_all[RP1:RP1 + 4, :], blk_ap(drC_d, 4, S, NH, rep=True))
            if flags["use_mask"]:
                nc.sync.dma_start(
                    lhs_all[KA:KA + 1, :],
                    bass.AP(tensor=m8_d.tensor, offset=0, ap=[[0, 1], [0, NH], [1, S]]))
                nc.vector.memset(rhs_all[KA:KA + 1, :], 1.0)

            wv16 = consts.tile([128, NT * S], F16, tag="wv16")
            nc.scalar.dma_start(wv16[:], blk_ap(wv_d, 128, S, NT))
            cret16 = consts.tile([128, NT * NF], F16, tag="cret16")
            nc.sync.dma_start(cret16[:], blk_ap(cret_d, 128, NF, NT))
            cimt16 = consts.tile([128, NT * NF], F16, tag="cimt16")
            nc.sync.dma_start(cimt16[:], blk_ap(cimt_d, 128, NF, NT))
            irA16 = consts.tile([128, 2 * S], F16, tag="irA16")
            nc.scalar.dma_start(irA16[:], blk_ap(irA_d, 128, S, 2))
            irB16 = consts.tile([128, 2 * S], F16, tag="irB16")
            nc.scalar.dma_start(irB16[:], blk_ap(irB_d, 128, S, 2))
            wrt16 = consts.tile([128, 2 * S], F16, tag="wrt16")
            nc.sync.dma_start(wrt16[:], blk_ap(wrt_d, 128, S, 2))
            wit16 = consts.tile([128, 2 * S], F16, tag="wit16")
            nc.sync.dma_start(wit16[:], blk_ap(wit_d, 128, S, 2))

            bias_cols = {}
            for nm, dd in (("bq", flags["use_bq"] and bq_d),
                           ("bk", flags["use_bk"] and bk_d)):
                if dd:
                    t = consts.tile([128, NT], F32, tag=nm)
                    nc.sync.dma_start(t[:], bass.AP(tensor=dd.tensor, offset=0,
                                                    ap=[[1, 128], [128, NT]]))
                    bias_cols[nm] = t
            if flags["use_bv"]:
                bv_row = consts.tile([1, H], F32, tag="bv")
                nc.sync.dma_start(bv_row[:], bass.AP(tensor=bv_d.tensor, offset=0,
                                                     ap=[[0, 1], [1, H]]))
            ln_bc = {}
            for nm, d_ap in ln_bcast.items():
                t = consts.tile([128, H], F32, tag=nm + "b")
                nc.gpsimd.dma_start(t[:], bass.AP(tensor=d_ap.tensor, offset=0,
                                                  ap=[[0, 128], [1, H]]))
                ln_bc[nm] = t

            # ---- small constants ----
            i16 = consts.tile([128, 128], F16, tag="i16")
            make_identity(nc, i16[:])
            negi16 = consts.tile([128, 128], F16, tag="negi16")
            nc.vector.tensor_scalar_mul(negi16[:], i16[:], -1.0)
            i32 = consts.tile([8, 8], F32, tag="i32")
            make_identity(nc, i32[:])
            i32r = consts.tile([8, 8], F32, tag="i32r")
            nc.vector.tensor_copy(i32r[:].bitcast(R), i32[:])
            onescol0 = consts.tile([1, 128], F32, tag="onescol0")
            nc.vector.memset(onescol0[:], 1.0)
            onescol = consts.tile([1, 128], F32, tag="onescol")
            nc.vector.tensor_copy(onescol[:].bitcast(R), onescol0[:])
            ones_f = consts.tile([128, NH], F32, tag="ones_f")
            nc.vector.memset(ones_f[:], 1.0)
            _ccols = {}

            def constcol(val):
                if val not in _ccols:
                    t = consts.tile([128, 1], F32, tag=f"cc{len(_ccols)}")
                    nc.vector.memset(t[:], val)
                    _ccols[val] = t
                return _ccols[val]

            # ---- prologue: X^T ----
            xt16 = []
            zpool = ctx.enter_context(tc.tile_pool(name="zpool", bufs=4))
            sppool = ctx.enter_context(tc.tile_pool(name="sppool", bufs=8))
            oqpool = ctx.enter_context(tc.tile_pool(name="oqpool", bufs=4))
            with ExitStack() as pctx:
                pA = pctx.enter_context(tc.tile_pool(name="pA", bufs=1, space="PSUM"))
                pB = pctx.enter_context(tc.tile_pool(name="pB", bufs=3, space="PSUM"))
                oqbp = pctx.enter_context(
                    tc.tile_pool(name="oqbp", bufs=2, space="PSUM"))
                for ht in range(NT):
                    tp = pA.tile([128, S], F16, tag="tp")
                    for st in range(NT):
                        nc.tensor.transpose(
                            tp[:, st * 128:(st + 1) * 128],
                            x16[:, st * S + ht * 128: st * S + (ht + 1) * 128],
                            i16[:])
                    t = consts.tile([128, S], F16, tag=f"xt{ht}", name=f"xt{ht}")
                    nc.vector.tensor_copy(t[:], tp[:])
                    if dbg and ht == 0:
                        nc.sync.dma_start(dbg_d["d_xt0"], t[:])
                    xt16.append(t)

                # ---- rows: oq/dq (q side), ok/dk (k side); separate base-0
                # PSUM groups per 8-row output ----
                def rows8(wblk, colbase, name, dt16, bias, eng):
                    psf = pB.tile([128, S], F32, tag="pj")
                    ps = psf[0:8, :]
                    for ht in range(NT):
                        nc.tensor.matmul(ps, wblk[:, ht * 16 + colbase:ht * 16 + colbase + 8],
                                         xt16[ht][:], start=(ht == 0),
                                         stop=(ht == NT - 1))
                    t = consts.tile([8, S], F16 if dt16 else F32, tag=name)
                    tout = t[:] if dt16 else t[:].bitcast(R)
                    if bias != 0.0:
                        nc.scalar.activation(tout, ps, AF.Identity,
                                             bias=constcol(float(bias))[0:8, 0:1],
                                             scale=1.0)
                    else:
                        eng.tensor_copy(tout, ps)
                    return t

                rows_oq = rows8(wblkq16[:], 0, "r_oq", False, c["b_order"], nc.vector)
                rows16_dq = rows8(wblkq16[:], 8, "r_dq", True, 0.0, nc.vector)
                rows_ok = rows8(wblkk16[:], 0, "r_ok", False, 0.0, nc.vector)
                rows16_dk = rows8(wblkk16[:], 8, "r_dk", True, c["b_dist"], nc.vector)

                # ok columns [128,8] per kt (z-build scalar ptr)
                okc = []
                for kt in range(NT):
                    ps = pB.tile([128, S], F32, tag="pj")
                    nc.tensor.matmul(ps[:, 0:8],
                                     rows_ok[:, kt * 128:(kt + 1) * 128].bitcast(R),
                                     i32r[:].bitcast(R), start=True, stop=True)
                    t = consts.tile([128, 8], F32, tag=f"okc{kt}")
                    nc.vector.tensor_copy(t[:], ps[:, 0:8])
                    okc.append(t)

                # ---- projections: q/k head rows into rhs_all/lhs_all ----
                for ot in range(NT):
                    psq = pB.tile([128, S], F32, tag="pj")
                    for ht in range(NT):
                        nc.tensor.matmul(psq[:], wq16[:, ht * S + ot * 128:ht * S + (ot + 1) * 128],
                                         xt16[ht][:], start=(ht == 0), stop=(ht == NT - 1))
                    psk = pB.tile([128, S], F32, tag="pj")
                    for ht in range(NT):
                        nc.tensor.matmul(psk[:], wk16[:, ht * S + ot * 128:ht * S + (ot + 1) * 128],
                                         xt16[ht][:], start=(ht == 0), stop=(ht == NT - 1))
                    for po, h in ((0, 2 * ot), (64, 2 * ot + 1)):
                        hsl = slice(h * S, (h + 1) * S)
                        if flags["use_bq"]:
                            nc.scalar.activation(rhs_all[0:KQ, hsl], psq[po:po + 64, :],
                                                 AF.Identity,
                                                 bias=bias_cols["bq"][po:po + 64, ot:ot + 1],
                                                 scale=1.0)
                        else:
                            nc.scalar.activation(rhs_all[0:KQ, hsl], psq[po:po + 64, :],
                                                 AF.Identity)
                        if flags["use_bk"]:
                            nc.scalar.activation(lhs_all[0:KQ, hsl], psk[po:po + 64, :],
                                                 AF.Identity,
                                                 bias=bias_cols["bk"][po:po + 64, ot:ot + 1],
                                                 scale=1.0)
                        else:
                            nc.scalar.activation(lhs_all[0:KQ, hsl], psk[po:po + 64, :],
                                                 AF.Identity)

                # ---- V projection -> vaug (value rows + ones column) ----
                vaug = []
                for st in range(NT):
                    ps = pB.tile([128, S], F32, tag="pj")
                    for ht in range(NT):
                        nc.tensor.matmul(ps[:], xt16[ht][:, st * 128:(st + 1) * 128],
                                         wv16[:, ht * S:(ht + 1) * S],
                                         start=(ht == 0),
                                         stop=(ht == NT - 1 and not flags["use_bv"]))
                    if flags["use_bv"]:
                        nc.tensor.matmul(ps[:], onescol[:].bitcast(R),
                                         bv_row[:].bitcast(R), start=False, stop=True)
                    t = consts.tile([128, NH * 65], F32, tag=f"vaug{st}", name=f"vaug{st}")
                    tap = t[:]
                    ones_cols = bass.AP(tensor=tap.tensor, offset=tap.offset + D,
                                        ap=[list(tap.ap[0]), [65, NH], [1, 1]])
                    nc.vector.tensor_copy(ones_cols.bitcast(R), ones_f[:])
                    dst = bass.AP(tensor=tap.tensor, offset=tap.offset,
                                  ap=[list(tap.ap[0]), [65, NH], [1, D]])
                    nc.vector.tensor_copy(dst.bitcast(R), ps[:])
                    vaug.append(t)
                # ---- oqb broadcast -> SBUF f32, then z + softplus per head ----
                sp16 = []
                for h in range(NH):
                    hsl = slice(h * S, (h + 1) * S)
                    dkb = pB.tile([128, S], F32, tag="pj")
                    nc.tensor.matmul(dkb[0:R1, :], sel16_t[:, h * R1:(h + 1) * R1],
                                     rows16_dk[:], start=True, stop=True)
                    nc.vector.scalar_tensor_tensor(
                        lhs_all[RS:RS + R1, hsl],
                        lhs_all[RU:RU + R1, hsl], twoc, dkb[0:R1, :],
                        op0=ALU.mult, op1=ALU.mult)
                    dqb = pB.tile([128, S], F32, tag="pj")
                    nc.tensor.matmul(dqb[0:R1, :], sel16_t[:, h * R1:(h + 1) * R1],
                                     rows16_dq[:], start=True, stop=True)
                    nc.vector.scalar_tensor_tensor(
                        rhs_all[RU:RU + R1, hsl],
                        rhs_all[RS:RS + R1, hsl], twoc, dqb[0:R1, :],
                        op0=ALU.mult, op1=ALU.mult)
                    ps = oqbp.tile([128, S], F32, tag="oqb")
                    nc.tensor.matmul(ps[:], onesel_t[:, h * 128:(h + 1) * 128],
                                     rows_oq[:].bitcast(R), start=True, stop=True)
                    oqt = oqpool.tile([128, S], F16, tag="oqbs")
                    nc.vector.tensor_copy(oqt[:], ps[:])
                    zt = zpool.tile([128, NT * S], F16, tag="z")
                    xw = zpool.tile([128, S], F16, tag="xw")
                    for kt in range(NT):
                        nc.vector.tensor_scalar(xw[:], oqt[:], okc[kt][:, h:h + 1],
                                                None, ALU.add)
                        nc.vector.tensor_tensor(zt[:, kt * S:(kt + 1) * S], xw[:],
                                                sst[:, kt * S:(kt + 1) * S], ALU.mult)
                    spt = sppool.tile([128, NT * S], F16, tag="sp")
                    if flags["use_softplus"]:
                        nc.scalar.activation(spt[:], zt[:], AF.Softplus)
                    else:
                        nc.scalar.activation(spt[:], zt[:], AF.Exp)
                        nc.scalar.activation(spt[:], spt[:], AF.Ln, bias=1.0, scale=1.0)
                    if dbg and h == 0:
                        nc.sync.dma_start(dbg_d["d_sp0"], spt[:])
                    sp16.append(spt)


            # ---- head loop ----
            ctxt16 = [consts.tile([128, S], F16, tag=f"ctxt{ht}", name=f"ctxt{ht}")
                      for ht in range(NT)]
            if dbg:
                nc.sync.dma_start(dbg_d["d_lhs"], lhs_all[0:KF, :])
                nc.sync.dma_start(dbg_d["d_rhs"], rhs_all[0:KF, :])
                nc.sync.dma_start(dbg_d["d_rows_oq"], rows_oq[:])
                nc.sync.dma_start(dbg_d["d_okc0"], okc[0][:])
            with ExitStack() as lctx:
                scp = lctx.enter_context(
                    tc.tile_pool(name="scp", bufs=2, space="PSUM"))
                ctxp = lctx.enter_context(
                    tc.tile_pool(name="ctxp", bufs=2, space="PSUM"))
                rbpp = lctx.enter_context(
                    tc.tile_pool(name="rbpp", bufs=1, space="PSUM"))
                for h in range(NH):
                    hb = h * S
                    et = etp.tile([128, NT * S], R, tag="et")
                    cps = ctxp.tile([65, S], F32, tag="cps")

                    def ctx_mm(kt):
                        nc.tensor.matmul(cps[:], vaug[kt][:, h * 65:(h + 1) * 65].bitcast(R),
                                         et[:, kt * S:(kt + 1) * S],
                                         start=(kt == 0), stop=(kt == NT - 1))

                    for half in range(2):
                        o = scp.tile([128, 2 * S], F32, tag="sc")
                        for kt in (2 * half, 2 * half + 1):
                            osl = o[:, (kt % 2) * S:(kt % 2) * S + S]
                            nc.tensor.matmul(osl, lhs_all[0:KF, hb + kt * 128:hb + (kt + 1) * 128],
                                             rhs_all[0:KF, hb:hb + S], start=True, stop=False)
                            nc.tensor.matmul(osl, negi16[:], sp16[h][:, kt * S:(kt + 1) * S],
                                             start=False, stop=True)
                        nc.scalar.activation(et[:, half * 2 * S:(half + 1) * 2 * S], o[:],
                                             AF.Exp, scale=0.125)
                        if half == 1:
                            ctx_mm(0)
                            ctx_mm(1)
                    ctx_mm(2)
                    ctx_mm(3)
                    # normalization: reciprocal of the denom row, broadcast via
                    # PE, fp16 copy + multiply on Pool
                    if dbg and h == 0:
                        etsb = work.tile([128, NT * S], F32, tag="dbget")
                        nc.vector.tensor_copy(etsb[:], et[:])
                        nc.sync.dma_start(dbg_d["d_et0"], etsb[:])
                        cpsb = work.tile([65, S], F32, tag="dbgcps")
                        nc.vector.tensor_copy(cpsb[:], cps[:])
                        nc.sync.dma_start(dbg_d["d_cps0"], cpsb[:])
                    rc = small.tile([1, S], mybir.dt.float32r, tag="rc")
                    with nc.allow_low_precision(reason="softmax denom reciprocal to f32r"):
                        nc.vector.reciprocal(rc[:], cps[64:65, :])
                    rbp = rbpp.tile([64, S], F32, tag="rbp")
                    nc.tensor.matmul(rbp[:], onescol[0:1, 0:64].bitcast(R), rc[:],
                                     start=True, stop=True)
                    rbs = work.tile([64, S], F16, tag="rbs")
                    nc.vector.tensor_copy(rbs[:], rbp[:])
                    po = (h % 2) * 64
                    nc.vector.tensor_tensor(ctxt16[h // 2][po:po + 64, :],
                                            cps[0:64, :], rbs[:], ALU.mult)

            if dbg:
                nc.sync.dma_start(dbg_d["d_ctxt0"], ctxt16[0][:])
            # ---- FFT filter + residual + layernorms (tail) ----
            with ExitStack() as fctx:
                fftp = fctx.enter_context(
                    tc.tile_pool(name="fftp", bufs=1, space="PSUM"))
                miscp = fctx.enter_context(
                    tc.tile_pool(name="miscp", bufs=2, space="PSUM"))
                pr16, pi16 = [], []
                for ft in range(2):
                    rt_ps = fftp.tile([128, S], F32, tag=f"rt{ft}")
                    it_ps = fftp.tile([128, S], F32, tag=f"it{ft}")
                    for ht in range(NT):
                        nc.tensor.matmul(rt_ps[:], cret16[:, ht * NF + ft * 128:ht * NF + (ft + 1) * 128],
                                         ctxt16[ht][:], start=(ht == 0), stop=(ht == NT - 1))
                    for ht in range(NT):
                        nc.tensor.matmul(it_ps[:], cimt16[:, ht * NF + ft * 128:ht * NF + (ft + 1) * 128],
                                         ctxt16[ht][:], start=(ht == 0), stop=(ht == NT - 1))
                    rts = work.tile([128, S], F16, tag="rts")
                    nc.vector.tensor_copy(rts[:], rt_ps[:])
                    its = work.tile([128, S], F16, tag="its")
                    nc.vector.tensor_copy(its[:], it_ps[:])
                    wrs = wrt16[:, ft * S:(ft + 1) * S]
                    wis = wit16[:, ft * S:(ft + 1) * S]
                    t1 = work.tile([128, S], F16, tag="f1")
                    t2 = work.tile([128, S], F16, tag="f2")
                    nc.gpsimd.tensor_tensor(t1[:], rts[:], wrs, ALU.mult)
                    nc.gpsimd.tensor_tensor(t2[:], its[:], wis, ALU.mult)
                    pr = consts.tile([128, S], F16, tag=f"pr{ft}", name=f"pr{ft}")
                    nc.vector.tensor_tensor(pr[:], t1[:], t2[:], ALU.subtract)
                    pr16.append(pr)
                    nc.gpsimd.tensor_tensor(t1[:], rts[:], wis, ALU.mult)
                    nc.vector.tensor_tensor(t2[:], its[:], wrs, ALU.mult)
                    pi = consts.tile([128, S], F16, tag=f"pi{ft}", name=f"pi{ft}")
                    nc.vector.tensor_tensor(pi[:], t1[:], t2[:], ALU.add)
                    pi16.append(pi)

                def layer_norm(dst, src, wname, bname, tagn):
                    st6 = small.tile([128, 6], F32, tag="st6" + tagn)
                    nc.vector.bn_stats(st6[:], src)
                    mv = small.tile([128, 2], F32, tag="mv" + tagn)
                    nc.vector.bn_aggr(mv[:], st6[:])
                    lnv = small.tile([128, 1], F32, tag="lnv" + tagn)
                    nc.scalar.activation(lnv[:], mv[:, 1:2], AF.Ln,
                                         bias=constcol(1e-12)[:, 0:1], scale=1.0)
                    rs = small.tile([128, 1], F32, tag="rs" + tagn)
                    nc.scalar.activation(rs[:], lnv[:], AF.Exp, scale=-0.5)
                    nb = small.tile([128, 1], F32, tag="nb" + tagn)
                    nc.vector.scalar_tensor_tensor(
                        nb[:], mv[:, 0:1], -1.0, rs[:],
                        op0=ALU.mult, op1=ALU.mult)
                    nc.scalar.activation(dst, src, AF.Identity,
                                         bias=nb[:, 0:1], scale=rs[:, 0:1])
                    if flags["use_" + wname]:
                        nc.vector.tensor_mul(dst, dst, ln_bc[wname][:])
                    if flags["use_" + bname]:
                        nc.vector.tensor_add(dst, dst, ln_bc[bname][:])

                for st in range(NT):
                    ssl = slice(st * 128, (st + 1) * 128)
                    yp = miscp.tile([128, S], F32, tag="yp")
                    for ft in range(2):
                        nc.tensor.matmul(yp[:], pr16[ft][:, ssl],
                                         irA16[:, ft * S:(ft + 1) * S],
                                         start=(ft == 0), stop=False)
                        nc.tensor.matmul(yp[:], pi16[ft][:, ssl],
                                         irB16[:, ft * S:(ft + 1) * S],
                                         start=False, stop=False)
                    for ht in range(NT):
                        nc.tensor.matmul(yp[:, ht * 128:(ht + 1) * 128],
                                         ctxt16[ht][:, ssl], i16[:],
                                         start=False, stop=(ht == NT - 1))
                    hid = work.tile([128, S], F32, tag="hid")
                    layer_norm(hid[:], yp[:], "lnfw", "lnfb", "a")
                    r2t = work.tile([128, S], F32, tag="r2")
                    nc.gpsimd.tensor_add(r2t[:], hid[:], x16[:, st * S:(st + 1) * S])
                    osb = work.tile([128, S], F32, tag="osb")
                    layer_norm(osb[:], r2t[:], "lnw", "lnb", "b")
                    nc.sync.dma_start(out_d[ssl, :], osb[:])

    nsplit = _split_excess_waits(nc)
    if nsplit:
        print(f"[kernel2] split {nsplit} excess sync waits onto NOPs")
    return nc


_CACHE = {}
LAST_EXEC_NS = None
LAST_RESULTS = None


def _flags_cvals(inputs):
    import os
    flags = {
        "use_mask": bool(np.any(inputs["attention_mask"] != 0)),
        "use_bq": bool(np.any(inputs["bq"] != 0)),
        "use_bk": bool(np.any(inputs["bk"] != 0)),
        "use_bv": bool(np.any(inputs["bv"] != 0)),
        "use_lnfw": not bool(np.all(inputs["ln_f_w"] == 1.0)),
        "use_lnfb": bool(np.any(inputs["ln_f_b"] != 0)),
        "use_lnw": not bool(np.all(inputs["ln_w"] == 1.0)),
        "use_lnb": bool(np.any(inputs["ln_b"] != 0)),
        "use_softplus": os.environ.get("KERNEL_SOFTPLUS", "") == "1",
    }
    cvals = {
        "c": float(inputs["scalar"][0]) ** 2 / 2.0,
        "b_order": float(inputs["b_order"][0]),
        "b_dist": float(inputs["b_dist"][0]),
    }
    return flags, cvals


def _shared_inputs(inputs, flags, cvals):
    hc = _host_constants()
    c = cvals["c"]
    Wq = inputs["Wq"].astype(np.float64)
    Wk = inputs["Wk"].astype(np.float64)
    wo, wd = inputs["W_order"].astype(np.float64), inputs["W_dist"].astype(np.float64)
    wblkq = np.zeros((H, 16), np.float64)
    wblkk = np.zeros((H, 16), np.float64)
    for h in range(NH):
        hs = slice(h * D, (h + 1) * D)
        wblkq[:, h] = Wq[:, hs] @ wo[:D, 0]
        wblkq[:, 8 + h] = Wq[:, hs] @ wd[:D, 0]
        wblkk[:, h] = Wk[:, hs] @ wo[D:, 0]
        wblkk[:, 8 + h] = Wk[:, hs] @ wd[D:, 0]
    cw = inputs["complex_weight"].astype(np.float32)
    shared = {
        "wq": inputs["Wq"].astype(np.float16),
        "wk": inputs["Wk"].astype(np.float16),
        "wv": inputs["Wv"].astype(np.float16),
        "wblkq": wblkq.astype(np.float16),
        "wblkk": wblkk.astype(np.float16),
        "ss": hc["ss"],
        "dlA": np.vstack([hc["U1"].T, hc["P2"].T[0:20]]).astype(np.float16),
        "dlB": hc["P2"].T[20:24].astype(np.float16),
        "drA": (-c * hc["S2c"].T[0:20]).astype(np.float16),
        "drB": hc["V1"].T.astype(np.float16),
        "drC": (-c * hc["S2c"].T[20:24]).astype(np.float16),
        "cret": hc["cret"], "cimt": hc["cimt"],
        "irA": hc["irA"], "irB": hc["irB"],
        "wrt": np.ascontiguousarray(cw[0, :, :NF, 0].T).astype(np.float16),
        "wit": np.ascontiguousarray(cw[0, :, :NF, 1].T).astype(np.float16),
        "onesel": hc["onesel"],
        "sel16": hc["sel16"],
    }
    if flags["use_bq"]:
        shared["bq"] = inputs["bq"].astype(np.float32)
    if flags["use_bk"]:
        shared["bk"] = inputs["bk"].astype(np.float32)
    if flags["use_bv"]:
        shared["bv"] = inputs["bv"].astype(np.float32)
    for nm, src in (("lnfw", "ln_f_w"), ("lnfb", "ln_f_b"),
                    ("lnw", "ln_w"), ("lnb", "ln_b")):
        if flags["use_" + nm]:
            shared[nm] = inputs[src].astype(np.float32)
    return shared


def kernel(**inputs):
    inputs = {k: np.asarray(v) for k, v in inputs.items()}
    x_all = inputs["input_tensor"].astype(np.float32)
    mask = inputs["attention_mask"].astype(np.float32)
    flags, cvals = _flags_cvals(inputs)

    key = (tuple(sorted(flags.items())), tuple(sorted(cvals.items())))
    if key not in _CACHE:
        _CACHE[key] = _build_program(cvals, flags)
    nc = _CACHE[key]

    shared = _shared_inputs(inputs, flags, cvals)
    in_maps = []
    for b in range(B):
        m = dict(shared)
        m["x"] = np.ascontiguousarray(x_all[b]).astype(np.float16)
        if flags["use_mask"]:
            m["m8"] = np.ascontiguousarray(8.0 * mask[b, 0, 0, :]).astype(np.float16)
        in_maps.append(m)

    import os
    trace = os.environ.get("KERNEL_TRACE", "") == "1"
    res = run_bass_kernel_spmd(nc, in_maps, core_ids=list(range(B)), trace=trace)
    global LAST_EXEC_NS, LAST_RESULTS
    LAST_RESULTS = res
    if res.exec_time_ns is not None:
        LAST_EXEC_NS = res.exec_time_ns
    out = np.stack([res.results[b]["out"] for b in range(B)]).astype(np.float32)
    return out


if __name__ == "__main__":
    print("kernel2 module ok")


# revision 3
# speedup vs baseline: 1.0578x; 1.0578x over previous
# Trainium2 Bass kernel for nn_Encoder_SelfAttention (sparse_attention), v2.
#
# Same contract as the baseline: kernel(**inputs) takes FULL unsharded inputs,
# shards batch across 8 cores, returns FULL (8,512,512) f32 output.
#
# v2 redesign (vs baseline at 144.5us):
# - Scores per (head, kt-block) built by ONE fused fp16 matmul with an
#   augmented K=112 contraction: rows 0..63 = q/k head rows (plain qk^T),
#   rows 64..87 = rank-24 SVD of -c*g^2, rows 88..111 = rank-12 SVD of g
#   paired with per-head 2c*dq / 2c*dk scaled basis rows. The tiny
#   -c*(dq+dk)^2 rank-1 terms (max 4e-3 in score units) are dropped.
#   PE matmul cost depends only on output columns, so folding all bias
#   terms into the contraction removes 4 of 5 score passes.
# - err_order: z = sign*(oq+ok) built by DVE/Pool stt into fp16; softplus as
#   a single ACT op (AF.Softplus, validated on hw; Exp+Ln fallback);
#   softplus SUBTRACTION done on PE via a -I fp16 matmul accumulated into
#   the scores PSUM group, so the final exp reads PSUM directly.
# - softmax denominator via ones-row in vaug (as baseline); reciprocal on
#   DVE (nc.vector.reciprocal), broadcast by a tiny PE matmul, and the
#   normalizing multiply on Pool (gpsimd) to keep DVE/ACT free.
# - FFT filter: fp16 DFT bases, Nyquist frequency dropped (validated
#   ~1e-3 end-to-end), filter products on DVE in fp16 (2x mode).
# - All big operands fp16 (weights, x for matmuls, sign matrix, bases):
#   halves DMA bytes; DMAs merged into one descriptor-batch per symbol.
# - z/softplus for ALL heads precomputed concurrently with projections so
#   the per-head PE stream (fused mm, -sp mm, ctx mm) never stalls on ACT.
import sys

sys.path.insert(0, "/opt/trn_rl_repo")

import numpy as np
from contextlib import ExitStack

import concourse.bass as bass
import concourse.tile as tile
from concourse import mybir
from concourse.bass_utils import run_bass_kernel_spmd
from concourse.masks import make_identity
from concourse.vector_clock import ScopedClock, VectorClock

F32 = mybir.dt.float32
F16 = mybir.dt.float16
R = mybir.dt.float32r
AF = mybir.ActivationFunctionType
ALU = mybir.AluOpType
B, S, H, NH, D = 8, 512, 512, 8, 64
NT = 4
R2, R1 = 24, 12          # SVD ranks for g^2 and g
KQ = 64                   # q/k head rows
KA = KQ + R2 + 2 * R1 + 3  # 115 fused contraction rows (116 with mask row)
NF = 256                  # kept rfft frequencies (Nyquist dropped)
# 32-aligned sub-blocks of the fused contraction (engine partition-start rule):
#   64..75  U (lhs, stt in0)      | V*2c*dq (rhs, stt out)
#   76..95  P[0:20]               | -c*S2[0:20]
#   96..107 U*2c*dk (lhs stt out) | V (rhs, stt in0)
#   108..111 P[20:24]             | -c*S2[20:24]
RU, RP0, RS, RP1 = 64, 76, 96, 108
# rows 112..114 (DMA-assembled, Taylor softplus): see _build_program
RT = 112


class _TileContext(tile.TileContext):
    # This walrus build rejects >1 sem wait on SP CTRL instructions; split
    # the tail-drain global-clock waits one-per-NOP. (Same as baseline.)
    def _drain_and_barrier(self, tick_clock, wait_clock):
        g = tick_clock.global_clock
        n = len(g)
        for i in range(n):
            if g[i] > 0:
                vec = [0] * n
                vec[i] = g[i]
                nop_inst = self.nc.sync.nop(nofuse=True)
                wait_clock.add_sem_waits(
                    nop_inst.ins, ScopedClock({None: VectorClock(vec)})
                )
        self.nc.sync.drain()
        self.nc.all_engine_barrier()
        assert self.sems is not None
        popped = self.nc._tile_sem_poison_stack.pop()
        assert popped is self._sem_poison
        self.nc.clear_and_free_semaphores(list(self.sems.allocated().values()))
        self.nc.all_engine_barrier()


def _split_excess_waits(nc):
    """Spill >cap sync-waits onto injected same-engine NOPs (walrus quirk)."""
    import bass_rust

    total = 0
    for fn in nc.m.functions:
        for blk in fn.blocks:
            out = []
            for inst in blk.instructions:
                si = inst.sync_info
                waits = list(si.on_wait) if si is not None else []
                cap = 2 if inst.__class__.__name__ == "InstEventSemaphore" else 1
                if len(waits) > cap:
                    keep, spill = waits[:cap], waits[cap:]
                    for w in spill:
                        nop = mybir.InstNoOp(
                            name=f"wsplit-{inst.name}-{total}", ins=[], outs=[])
                        nop.engine = inst.engine
                        nop.sync_info = bass_rust.SyncInfo(on_wait=[w], on_update=[])
                        out.append(nop)
                        total += 1
                    inst.sync_info = bass_rust.SyncInfo(
                        on_wait=keep, on_update=list(si.on_update))
                out.append(inst)
            blk.instructions = out
    return total


_HC = None


def _host_constants():
    """Input-independent structural constants (cached)."""
    global _HC
    if _HC is not None:
        return _HC
    idx = np.arange(S)
    g = np.log(np.abs(idx[None, :] - idx[:, None]).astype(np.float64) + 1.0)
    g2 = g ** 2
    u2, s2, vt2 = np.linalg.svd(g2)
    P2 = u2[:, :R2] * np.sqrt(s2[:R2])
    S2c = vt2[:R2].T * np.sqrt(s2[:R2])          # g2 ~= P2 @ S2c.T
    u1, s1, vt1 = np.linalg.svd(g)
    U1 = u1[:, :R1] * np.sqrt(s1[:R1])
    V1 = vt1[:R1].T * np.sqrt(s1[:R1])           # g ~= U1 @ V1.T
    # rfft/irfft ortho bases, Nyquist (freq 256) dropped
    W = np.fft.rfft(np.eye(H), norm="ortho", axis=-1)
    cret = np.ascontiguousarray(W.real[:, :NF]).astype(np.float16)   # [H, NF]
    cimt = np.ascontiguousarray(W.imag[:, :NF]).astype(np.float16)
    irA = np.fft.irfft(np.eye(257), n=H, norm="ortho", axis=-1)[:NF].astype(np.float16)
    irB = np.fft.irfft(1j * np.eye(257), n=H, norm="ortho", axis=-1)[:NF].astype(np.float16)
    Lm = np.where(idx[:, None] > idx[None, :], 0.5, -0.5).astype(np.float16)  # [k,q]
    sel16 = np.zeros((NH, NH * R1), np.float16)  # dk/dq head-row selector
    for h in range(NH):
        sel16[h, h * R1:(h + 1) * R1] = 1.0
    _HC = dict(g=g, g2=g2, P2=P2, S2c=S2c, U1=U1, V1=V1,
               cret=cret, cimt=cimt, irA=irA, irB=irB,
               Lm=Lm, LmT=np.ascontiguousarray(Lm.T), sel16=sel16)
    return _HC


def _build_program(c, flags):
    hc = _host_constants()
    nc = bass.Bass("TRN2", target_bir_lowering=False, debug=False)
    negc = -c["c"]
    twoc = 2.0 * c["c"]
    KF = KA + 1 if flags["use_mask"] else KA   # fused contraction depth

    def din(name, shape, dt):
        return nc.dram_tensor(name, list(shape), dt, kind="ExternalInput").ap()

    x_d = din("x", (S, H), F16)
    wq_d = din("wq", (H, H), F16)
    wk_d = din("wk", (H, H), F16)
    wv_d = din("wv", (H, H), F16)
    wblkq_d = din("wblkq", (H, 16), F16)
    wblkk_d = din("wblkk", (H, 16), F16)
    lm_d = din("lm", (S, S), F16)
    lmt_d = din("lmt", (S, S), F16)
    ones_d = din("ones1", (1, S), F16)
    dlA_d = din("dlA", (32, S), F16)         # [U1^T ; P2^T[0:20]]
    dlB_d = din("dlB", (4, S), F16)          # P2^T[20:24]
    drA_d = din("drA", (20, S), F16)         # -c*S2c^T[0:20]
    drB_d = din("drB", (R1, S), F16)         # V1^T
    drC_d = din("drC", (4, S), F16)          # -c*S2c^T[20:24]
    cret_d = din("cret", (H, NF), F16)
    cimt_d = din("cimt", (H, NF), F16)
    irA_d = din("irA", (NF, H), F16)
    irB_d = din("irB", (NF, H), F16)
    wrt_d = din("wrt", (NF, S), F16)
    wit_d = din("wit", (NF, S), F16)
    sel16_d = din("sel16", (NH, NH * R1), F16)
    if flags["use_mask"]:
        m8_d = din("m8", (S,), F16)
    if flags["use_bq"]:
        bq_d = din("bq", (H,), F32)
    if flags["use_bk"]:
        bk_d = din("bk", (H,), F32)
    if flags["use_bv"]:
        bv_d = din("bv", (H,), F32)
    ln_bcast = {}
    for nm in ("lnfw", "lnfb", "lnw", "lnb"):
        if flags["use_" + nm]:
            ln_bcast[nm] = din(nm, (H,), F32)
    out_d = nc.dram_tensor("out", [S, H], F32, kind="ExternalOutput").ap()
    import os
    dbg = os.environ.get("KERNEL_DEBUG", "") == "1"
    dbg_d = {}
    if dbg:
        for nm, shape, dt in (("d_xt0", (128, S), F16), ("d_lhs", (KF, NH * S), F16),
                              ("d_rhs", (KF, NH * S), F16),
                              ("d_et0", (128, NT * S), F32), ("d_cps0", (65, S), F32),
                              ("d_ctxt0", (128, S), F16), ("d_rows_oq", (8, S), F32),
                              ("d_okc0", (128, 8), F32)):
            dbg_d[nm] = nc.dram_tensor(nm, list(shape), dt, kind="ExternalOutput").ap()

    def blk_ap(d, rows, width, nblk, rep=False):
        """3D ap: HBM [rows*nblk, width] -> SBUF [rows, nblk*width].
        rep=True re-reads the same [rows,width] block nblk times."""
        return bass.AP(tensor=d.tensor, offset=0,
                       ap=[[width, rows], [0 if rep else rows * width, nblk],
                           [1, width]])

    def rep_load(engine, dst_tile_slice, d, rows, width, nblk):
        """Replicated load as nblk separate DMAs (no zero-stride free dim)."""
        ap0 = dst_tile_slice
        for b in range(nblk):
            sub = bass.AP(tensor=ap0.tensor, offset=ap0.offset + b * width,
                          ap=[list(ap0.ap[0]), [1, width]])
            engine.dma_start(sub, bass.AP(tensor=d.tensor, offset=0,
                                          ap=[[width, rows], [1, width]]))

    with _TileContext(nc) as tc:
        with ExitStack() as ctx:
            consts = ctx.enter_context(tc.tile_pool(name="consts", bufs=1))
            work = ctx.enter_context(tc.tile_pool(name="work", bufs=2))
            etp = ctx.enter_context(tc.tile_pool(name="etp", bufs=2))
            small = ctx.enter_context(tc.tile_pool(name="small", bufs=2))

            # ---- DMA loads (merged, ordered by first use) ----
            x16 = consts.tile([128, NT * S], F16, tag="x16")
            nc.sync.dma_start(x16[:, 0:2 * S],
                              bass.AP(tensor=x_d.tensor, offset=0,
                                      ap=[[S, 128], [128 * S, 2], [1, S]]))
            nc.sync.dma_start(x16[:, 2 * S:],
                              bass.AP(tensor=x_d.tensor, offset=2 * 128 * S,
                                      ap=[[S, 128], [128 * S, 2], [1, S]]))
            wq16 = consts.tile([128, NT * S], F16, tag="wq16")
            nc.scalar.dma_start(wq16[:], blk_ap(wq_d, 128, S, NT))
            wk16 = consts.tile([128, NT * S], F16, tag="wk16")
            nc.sync.dma_start(wk16[:], blk_ap(wk_d, 128, S, NT))
            wblkq16 = consts.tile([128, NT * 16], F16, tag="wblkq16")
            nc.scalar.dma_start(wblkq16[:], blk_ap(wblkq_d, 128, 16, NT))
            wblkk16 = consts.tile([128, NT * 16], F16, tag="wblkk16")
            nc.scalar.dma_start(wblkk16[:], blk_ap(wblkk_d, 128, 16, NT))
            lm_t = consts.tile([128, NT * S], F16, tag="lm")
            nc.sync.dma_start(lm_t[:], blk_ap(lm_d, 128, S, NT))
            lmt_t = consts.tile([128, NT * S], F16, tag="lmt")
            nc.sync.dma_start(lmt_t[:], blk_ap(lmt_d, 128, S, NT))
            sel16_t = consts.tile([NH, NH * R1], F16, tag="sel16")
            nc.sync.dma_start(sel16_t[:], sel16_d)

            # Fused-contraction operand tiles; aug rows replicated x8 by DMA
            lhs_all = consts.tile([KF, NH * S], F16, tag="lhs_all", name="lhs_all")
            rhs_all = consts.tile([KF, NH * S], F16, tag="rhs_all", name="rhs_all")
            nc.scalar.dma_start(lhs_all[RU:RU + 32, :], blk_ap(dlA_d, 32, S, NH, rep=True))
            nc.scalar.dma_start(lhs_all[RP1:RP1 + 4, :], blk_ap(dlB_d, 4, S, NH, rep=True))
            nc.sync.dma_start(rhs_all[RP0:RP0 + 20, :], blk_ap(drA_d, 20, S, NH, rep=True))
            nc.sync.dma_start(rhs_all[RS:RS + R1, :], blk_ap(drB_d, R1, S, NH, rep=True))
            nc.sync.dma_start(rhs_all[RP1:RP1 + 4, :], blk_ap(drC_d, 4, S, NH, rep=True))
            # Taylor rows 113(lhs)/114(rhs) are all-ones (host replicated)
            nc.scalar.dma_start(lhs_all[RT + 2:RT + 3, :],
                                blk_ap(ones_d, 1, S, NH, rep=True))
            nc.scalar.dma_start(rhs_all[RT + 1:RT + 2, :],
                                blk_ap(ones_d, 1, S, NH, rep=True))
            if flags["use_mask"]:
                nc.sync.dma_start(
                    lhs_all[KA:KA + 1, :],
                    bass.AP(tensor=m8_d.tensor, offset=0, ap=[[0, 1], [0, NH], [1, S]]))
                nc.scalar.dma_start(rhs_all[KA:KA + 1, :],
                                    blk_ap(ones_d, 1, S, NH, rep=True))

            wv16 = consts.tile([128, NT * S], F16, tag="wv16")
            nc.scalar.dma_start(wv16[:], blk_ap(wv_d, 128, S, NT))
            cret16 = consts.tile([128, NT * NF], F16, tag="cret16")
            nc.sync.dma_start(cret16[:], blk_ap(cret_d, 128, NF, NT))
            cimt16 = consts.tile([128, NT * NF], F16, tag="cimt16")
            nc.sync.dma_start(cimt16[:], blk_ap(cimt_d, 128, NF, NT))
            irA16 = consts.tile([128, 2 * S], F16, tag="irA16")
            nc.scalar.dma_start(irA16[:], blk_ap(irA_d, 128, S, 2))
            irB16 = consts.tile([128, 2 * S], F16, tag="irB16")
            nc.scalar.dma_start(irB16[:], blk_ap(irB_d, 128, S, 2))
            wrt16 = consts.tile([128, 2 * S], F16, tag="wrt16")
            nc.sync.dma_start(wrt16[:], blk_ap(wrt_d, 128, S, 2))
            wit16 = consts.tile([128, 2 * S], F16, tag="wit16")
            nc.sync.dma_start(wit16[:], blk_ap(wit_d, 128, S, 2))

            bias_cols = {}
            for nm, dd in (("bq", flags["use_bq"] and bq_d),
                           ("bk", flags["use_bk"] and bk_d)):
                if dd:
                    t = consts.tile([128, NT], F32, tag=nm)
                    nc.sync.dma_start(t[:], bass.AP(tensor=dd.tensor, offset=0,
                                                    ap=[[1, 128], [128, NT]]))
                    bias_cols[nm] = t
            if flags["use_bv"]:
                bv_row = consts.tile([1, H], F32, tag="bv")
                nc.sync.dma_start(bv_row[:], bass.AP(tensor=bv_d.tensor, offset=0,
                                                     ap=[[0, 1], [1, H]]))
            ln_bc = {}
            for nm, d_ap in ln_bcast.items():
                t = consts.tile([128, H], F32, tag=nm + "b")
                nc.gpsimd.dma_start(t[:], bass.AP(tensor=d_ap.tensor, offset=0,
                                                  ap=[[0, 128], [1, H]]))
                ln_bc[nm] = t

            # ---- small constants ----
            i16 = consts.tile([128, 128], F16, tag="i16")
            make_identity(nc, i16[:])
            i32 = consts.tile([8, 8], F32, tag="i32")
            make_identity(nc, i32[:])
            i32r = consts.tile([8, 8], F32, tag="i32r")
            nc.vector.tensor_copy(i32r[:].bitcast(R), i32[:])
            onescol0 = consts.tile([1, 128], F32, tag="onescol0")
            nc.vector.memset(onescol0[:], 1.0)
            onescol = consts.tile([1, 128], F32, tag="onescol")
            nc.vector.tensor_copy(onescol[:].bitcast(R), onescol0[:])
            ones_f = consts.tile([128, NH], F32, tag="ones_f")
            nc.vector.memset(ones_f[:], 1.0)
            _ccols = {}

            def constcol(val):
                if val not in _ccols:
                    t = consts.tile([128, 1], F32, tag=f"cc{len(_ccols)}")
                    nc.vector.memset(t[:], val)
                    _ccols[val] = t
                return _ccols[val]

            # ---- prologue: X^T ----
            xt16 = []
            diagp = ctx.enter_context(tc.tile_pool(name="diagp", bufs=6))
            with ExitStack() as pctx:
                pA = pctx.enter_context(tc.tile_pool(name="pA", bufs=1, space="PSUM"))
                pB = pctx.enter_context(tc.tile_pool(name="pB", bufs=3, space="PSUM"))
                for ht in range(NT):
                    tp = pA.tile([128, S], F16, tag="tp")
                    for st in range(NT):
                        nc.tensor.transpose(
                            tp[:, st * 128:(st + 1) * 128],
                            x16[:, st * S + ht * 128: st * S + (ht + 1) * 128],
                            i16[:])
                    t = consts.tile([128, S], F16, tag=f"xt{ht}", name=f"xt{ht}")
                    nc.vector.tensor_copy(t[:], tp[:])
                    if dbg and ht == 0:
                        nc.sync.dma_start(dbg_d["d_xt0"], t[:])
                    xt16.append(t)

                # ---- rows: oq/dq (q side), ok/dk (k side); separate base-0
                # PSUM groups per 8-row output ----
                def rows8(wblk, colbase, name, dt16, bias, eng):
                    psf = pB.tile([128, S], F32, tag="pj")
                    ps = psf[0:8, :]
                    for ht in range(NT):
                        nc.tensor.matmul(ps, wblk[:, ht * 16 + colbase:ht * 16 + colbase + 8],
                                         xt16[ht][:], start=(ht == 0),
                                         stop=(ht == NT - 1))
                    t = consts.tile([8, S], F16 if dt16 else F32, tag=name)
                    tout = t[:] if dt16 else t[:].bitcast(R)
                    if bias != 0.0:
                        nc.scalar.activation(tout, ps, AF.Identity,
                                             bias=constcol(float(bias))[0:8, 0:1],
                                             scale=1.0)
                    else:
                        eng.tensor_copy(tout, ps)
                    return t

                rows_oq = rows8(wblkq16[:], 0, "r_oq", False, c["b_order"], nc.vector)
                rows16_dq = rows8(wblkq16[:], 8, "r_dq", True, 0.0, nc.vector)
                rows_ok = rows8(wblkk16[:], 0, "r_ok", False, 0.0, nc.vector)
                rows16_dk = rows8(wblkk16[:], 8, "r_dk", True, c["b_dist"], nc.vector)

                # ok/oq columns [128,8] per block (diag-build scalar ptrs)
                okc, oqc = [], []
                for kt in range(NT):
                    ps = pB.tile([128, S], F32, tag="pj")
                    nc.tensor.matmul(ps[:, 0:8],
                                     rows_ok[:, kt * 128:(kt + 1) * 128].bitcast(R),
                                     i32r[:].bitcast(R), start=True, stop=False)
                    nc.tensor.matmul(ps[:, 8:16],
                                     rows_oq[:, kt * 128:(kt + 1) * 128].bitcast(R),
                                     i32r[:].bitcast(R), start=False, stop=True)
                    t = consts.tile([128, 16], F32, tag=f"okc{kt}")
                    nc.vector.tensor_copy(t[:], ps[:, 0:16])
                    okc.append(t)
                    oqc.append(t)

                # Taylor staging rows (all heads): w'=-ok^2/8, u'=-oq^2/8,
                # -oq/4, ok (fp16), then flat SBUF->SBUF DMAs scatter them
                # into aug rows 112..114 of each head block.
                ok16s = consts.tile([8, S], F16, tag="ok16s")
                nc.vector.tensor_copy(ok16s[:], rows_ok[:])
                sqk = consts.tile([8, S], F16, tag="sqk")
                nc.vector.tensor_tensor(sqk[:], rows_ok[:], rows_ok[:], ALU.mult)
                w16s = consts.tile([8, S], F16, tag="w16s")
                nc.vector.tensor_scalar(w16s[:], sqk[:], -0.125, None, ALU.mult)
                tq16s = consts.tile([8, S], F16, tag="tq16s")
                nc.vector.tensor_scalar(tq16s[:], rows_oq[:], -0.25, None, ALU.mult)
                squ = consts.tile([8, S], F16, tag="squ")
                nc.vector.tensor_tensor(squ[:], rows_oq[:], rows_oq[:], ALU.mult)
                u16s = consts.tile([8, S], F16, tag="u16s")
                nc.vector.tensor_scalar(u16s[:], squ[:], -0.125, None, ALU.mult)

                def flat_row(dst_row, srct):
                    nc.sync.dma_start(dst_row, srct[:])

                flat_row(lhs_all[RT:RT + 1, :], ok16s)       # ok  | -oq/4
                flat_row(rhs_all[RT:RT + 1, :], tq16s)
                flat_row(lhs_all[RT + 1:RT + 2, :], w16s)    # w'  | ones
                flat_row(rhs_all[RT + 2:RT + 3, :], u16s)    # ones| u' 

                # ---- projections: q/k head rows into rhs_all/lhs_all ----
                for ot in range(NT):
                    psq = pB.tile([128, S], F32, tag="pj")
                    for ht in range(NT):
                        nc.tensor.matmul(psq[:], wq16[:, ht * S + ot * 128:ht * S + (ot + 1) * 128],
                                         xt16[ht][:], start=(ht == 0), stop=(ht == NT - 1))
                    psk = pB.tile([128, S], F32, tag="pj")
                    for ht in range(NT):
                        nc.tensor.matmul(psk[:], wk16[:, ht * S + ot * 128:ht * S + (ot + 1) * 128],
                                         xt16[ht][:], start=(ht == 0), stop=(ht == NT - 1))
                    for po, h in ((0, 2 * ot), (64, 2 * ot + 1)):
                        hsl = slice(h * S, (h + 1) * S)
                        if flags["use_bq"]:
                            nc.scalar.activation(rhs_all[0:KQ, hsl], psq[po:po + 64, :],
                                                 AF.Identity,
                                                 bias=bias_cols["bq"][po:po + 64, ot:ot + 1],
                                                 scale=1.0)
                        else:
                            nc.scalar.activation(rhs_all[0:KQ, hsl], psq[po:po + 64, :],
                                                 AF.Identity)
                        if flags["use_bk"]:
                            nc.scalar.activation(lhs_all[0:KQ, hsl], psk[po:po + 64, :],
                                                 AF.Identity,
                                                 bias=bias_cols["bk"][po:po + 64, ot:ot + 1],
                                                 scale=1.0)
                        else:
                            nc.scalar.activation(lhs_all[0:KQ, hsl], psk[po:po + 64, :],
                                                 AF.Identity)

                # ---- V projection -> vaug (value rows + ones column) ----
                vaug = []
                for st in range(NT):
                    ps = pB.tile([128, S], F32, tag="pj")
                    for ht in range(NT):
                        nc.tensor.matmul(ps[:], xt16[ht][:, st * 128:(st + 1) * 128],
                                         wv16[:, ht * S:(ht + 1) * S],
                                         start=(ht == 0),
                                         stop=(ht == NT - 1 and not flags["use_bv"]))
                    if flags["use_bv"]:
                        nc.tensor.matmul(ps[:], onescol[:].bitcast(R),
                                         bv_row[:].bitcast(R), start=False, stop=True)
                    t = consts.tile([128, NH * 65], F32, tag=f"vaug{st}", name=f"vaug{st}")
                    tap = t[:]
                    ones_cols = bass.AP(tensor=tap.tensor, offset=tap.offset + D,
                                        ap=[list(tap.ap[0]), [65, NH], [1, 1]])
                    nc.vector.tensor_copy(ones_cols.bitcast(R), ones_f[:])
                    dst = bass.AP(tensor=tap.tensor, offset=tap.offset,
                                  ap=[list(tap.ap[0]), [65, NH], [1, D]])
                    nc.vector.tensor_copy(dst.bitcast(R), ps[:])
                    vaug.append(t)
                # ---- per-head scaled SVD basis rows ----
                for h in range(NH):
                    hsl = slice(h * S, (h + 1) * S)
                    dkb = pB.tile([128, S], F32, tag="pj")
                    nc.tensor.matmul(dkb[0:R1, :], sel16_t[:, h * R1:(h + 1) * R1],
                                     rows16_dk[:], start=True, stop=True)
                    nc.vector.scalar_tensor_tensor(
                        lhs_all[RS:RS + R1, hsl],
                        lhs_all[RU:RU + R1, hsl], twoc, dkb[0:R1, :],
                        op0=ALU.mult, op1=ALU.mult)
                    dqb = pB.tile([128, S], F32, tag="pj")
                    nc.tensor.matmul(dqb[0:R1, :], sel16_t[:, h * R1:(h + 1) * R1],
                                     rows16_dq[:], start=True, stop=True)
                    nc.vector.scalar_tensor_tensor(
                        rhs_all[RU:RU + R1, hsl],
                        rhs_all[RS:RS + R1, hsl], twoc, dqb[0:R1, :],
                        op0=ALU.mult, op1=ALU.mult)

            # ---- head loop ----
            ctxt16 = [consts.tile([128, S], F16, tag=f"ctxt{ht}", name=f"ctxt{ht}")
                      for ht in range(NT)]
            if dbg:
                nc.sync.dma_start(dbg_d["d_lhs"], lhs_all[0:KF, :])
                nc.sync.dma_start(dbg_d["d_rhs"], rhs_all[0:KF, :])
                nc.sync.dma_start(dbg_d["d_rows_oq"], rows_oq[:])
                nc.sync.dma_start(dbg_d["d_okc0"], okc[0][:])
            with ExitStack() as lctx:
                scp = lctx.enter_context(
                    tc.tile_pool(name="scp", bufs=2, space="PSUM"))
                ctxp = lctx.enter_context(
                    tc.tile_pool(name="ctxp", bufs=2, space="PSUM"))
                rbpp = lctx.enter_context(
                    tc.tile_pool(name="rbpp", bufs=1, space="PSUM"))
                for h in range(NH):
                    hb = h * S
                    et = etp.tile([128, NT * S], R, tag="et")
                    cps = ctxp.tile([65, S], F32, tag="cps")

                    def ctx_mm(kt):
                        nc.tensor.matmul(cps[:], vaug[kt][:, h * 65:(h + 1) * 65].bitcast(R),
                                         et[:, kt * S:(kt + 1) * S],
                                         start=(kt == 0), stop=(kt == NT - 1))

                    doq = []
                    for qt in range(NT):
                        dt_ = diagp.tile([128, 128], F16, tag="doq")
                        nc.gpsimd.tensor_scalar_mul(dt_[:], i16[:],
                                                    oqc[qt][:, 8 + h:9 + h])
                        doq.append(dt_)
                    for half in range(2):
                        o = scp.tile([128, 2 * S], F32, tag="sc")
                        for kt in (2 * half, 2 * half + 1):
                            osl = o[:, (kt % 2) * S:(kt % 2) * S + S]
                            nc.tensor.matmul(osl, lhs_all[0:KF, hb + kt * 128:hb + (kt + 1) * 128],
                                             rhs_all[0:KF, hb:hb + S], start=True, stop=False)
                            dok = diagp.tile([128, 128], F16, tag="dok")
                            nc.gpsimd.tensor_scalar_mul(dok[:], i16[:],
                                                        okc[kt][:, h:h + 1])
                            nc.tensor.matmul(osl, dok[:], lm_t[:, kt * S:(kt + 1) * S],
                                             start=False, stop=False)
                            for qt in range(NT):
                                nc.tensor.matmul(
                                    osl[:, qt * 128:(qt + 1) * 128],
                                    lmt_t[:, qt * S + kt * 128:qt * S + (kt + 1) * 128],
                                    doq[qt][:], start=False, stop=(qt == NT - 1))
                        nc.scalar.activation(et[:, half * 2 * S:(half + 1) * 2 * S], o[:],
                                             AF.Exp, scale=0.125)
                        if half == 1:
                            ctx_mm(0)
                            ctx_mm(1)
                    ctx_mm(2)
                    ctx_mm(3)
                    # normalization: reciprocal of the denom row, broadcast via
                    # PE, fp16 copy + multiply on Pool
                    if dbg and h == 0:
                        etsb = work.tile([128, NT * S], F32, tag="dbget")
                        nc.vector.tensor_copy(etsb[:], et[:])
                        nc.sync.dma_start(dbg_d["d_et0"], etsb[:])
                        cpsb = work.tile([65, S], F32, tag="dbgcps")
                        nc.vector.tensor_copy(cpsb[:], cps[:])
                        nc.sync.dma_start(dbg_d["d_cps0"], cpsb[:])
                    rc = small.tile([1, S], mybir.dt.float32r, tag="rc")
                    with nc.allow_low_precision(reason="softmax denom reciprocal to f32r"):
                        nc.vector.reciprocal(rc[:], cps[64:65, :])
                    rbp = rbpp.tile([64, S], F32, tag="rbp")
                    nc.tensor.matmul(rbp[:], onescol[0:1, 0:64].bitcast(R), rc[:],
                                     start=True, stop=True)
                    rbs = work.tile([64, S], F16, tag="rbs")
                    nc.vector.tensor_copy(rbs[:], rbp[:])
                    po = (h % 2) * 64
                    nc.vector.tensor_tensor(ctxt16[h // 2][po:po + 64, :],
                                            cps[0:64, :], rbs[:], ALU.mult)

            if dbg:
                nc.sync.dma_start(dbg_d["d_ctxt0"], ctxt16[0][:])
            # ---- FFT filter + residual + layernorms (tail) ----
            with ExitStack() as fctx:
                fftp = fctx.enter_context(
                    tc.tile_pool(name="fftp", bufs=1, space="PSUM"))
                miscp = fctx.enter_context(
                    tc.tile_pool(name="miscp", bufs=2, space="PSUM"))
                pr16, pi16 = [], []
                for ft in range(2):
                    rt_ps = fftp.tile([128, S], F32, tag=f"rt{ft}")
                    it_ps = fftp.tile([128, S], F32, tag=f"it{ft}")
                    for ht in range(NT):
                        nc.tensor.matmul(rt_ps[:], cret16[:, ht * NF + ft * 128:ht * NF + (ft + 1) * 128],
                                         ctxt16[ht][:], start=(ht == 0), stop=(ht == NT - 1))
                    for ht in range(NT):
                        nc.tensor.matmul(it_ps[:], cimt16[:, ht * NF + ft * 128:ht * NF + (ft + 1) * 128],
                                         ctxt16[ht][:], start=(ht == 0), stop=(ht == NT - 1))
                    rts = work.tile([128, S], F16, tag="rts")
                    nc.vector.tensor_copy(rts[:], rt_ps[:])
                    its = work.tile([128, S], F16, tag="its")
                    nc.vector.tensor_copy(its[:], it_ps[:])
                    wrs = wrt16[:, ft * S:(ft + 1) * S]
                    wis = wit16[:, ft * S:(ft + 1) * S]
                    t1 = work.tile([128, S], F16, tag="f1")
                    t2 = work.tile([128, S], F16, tag="f2")
                    nc.gpsimd.tensor_tensor(t1[:], rts[:], wrs, ALU.mult)
                    nc.gpsimd.tensor_tensor(t2[:], its[:], wis, ALU.mult)
                    pr = consts.tile([128, S], F16, tag=f"pr{ft}", name=f"pr{ft}")
                    nc.vector.tensor_tensor(pr[:], t1[:], t2[:], ALU.subtract)
                    pr16.append(pr)
                    nc.gpsimd.tensor_tensor(t1[:], rts[:], wis, ALU.mult)
                    nc.vector.tensor_tensor(t2[:], its[:], wrs, ALU.mult)
                    pi = consts.tile([128, S], F16, tag=f"pi{ft}", name=f"pi{ft}")
                    nc.vector.tensor_tensor(pi[:], t1[:], t2[:], ALU.add)
                    pi16.append(pi)

                def layer_norm(dst, src, wname, bname, tagn):
                    st6 = small.tile([128, 6], F32, tag="st6" + tagn)
                    nc.vector.bn_stats(st6[:], src)
                    mv = small.tile([128, 2], F32, tag="mv" + tagn)
                    nc.vector.bn_aggr(mv[:], st6[:])
                    lnv = small.tile([128, 1], F32, tag="lnv" + tagn)
                    nc.scalar.activation(lnv[:], mv[:, 1:2], AF.Ln,
                                         bias=constcol(1e-12)[:, 0:1], scale=1.0)
                    rs = small.tile([128, 1], F32, tag="rs" + tagn)
                    nc.scalar.activation(rs[:], lnv[:], AF.Exp, scale=-0.5)
                    nb = small.tile([128, 1], F32, tag="nb" + tagn)
                    nc.vector.scalar_tensor_tensor(
                        nb[:], mv[:, 0:1], -1.0, rs[:],
                        op0=ALU.mult, op1=ALU.mult)
                    nc.scalar.activation(dst, src, AF.Identity,
                                         bias=nb[:, 0:1], scale=rs[:, 0:1])
                    if flags["use_" + wname]:
                        nc.vector.tensor_mul(dst, dst, ln_bc[wname][:])
                    if flags["use_" + bname]:
                        nc.vector.tensor_add(dst, dst, ln_bc[bname][:])

                for st in range(NT):
                    ssl = slice(st * 128, (st + 1) * 128)
                    yp = miscp.tile([128, S], F32, tag="yp")
                    for ft in range(2):
                        nc.tensor.matmul(yp[:], pr16[ft][:, ssl],
                                         irA16[:, ft * S:(ft + 1) * S],
                                         start=(ft == 0), stop=False)
                        nc.tensor.matmul(yp[:], pi16[ft][:, ssl],
                                         irB16[:, ft * S:(ft + 1) * S],
                                         start=False, stop=False)
                    for ht in range(NT):
                        nc.tensor.matmul(yp[:, ht * 128:(ht + 1) * 128],
                                         ctxt16[ht][:, ssl], i16[:],
                                         start=False, stop=(ht == NT - 1))
                    hid = work.tile([128, S], F32, tag="hid")
                    layer_norm(hid[:], yp[:], "lnfw", "lnfb", "a")
                    r2t = work.tile([128, S], F32, tag="r2")
                    nc.gpsimd.tensor_add(r2t[:], hid[:], x16[:, st * S:(st + 1) * S])
                    osb = work.tile([128, S], F32, tag="osb")
                    layer_norm(osb[:], r2t[:], "lnw", "lnb", "b")
                    nc.sync.dma_start(out_d[ssl, :], osb[:])

    nsplit = _split_excess_waits(nc)
    if nsplit:
        print(f"[kernel2] split {nsplit} excess sync waits onto NOPs")
    return nc


_CACHE = {}
LAST_EXEC_NS = None
LAST_RESULTS = None


def _flags_cvals(inputs):
    import os
    flags = {
        "use_mask": bool(np.any(inputs["attention_mask"] != 0)),
        "use_bq": bool(np.any(inputs["bq"] != 0)),
        "use_bk": bool(np.any(inputs["bk"] != 0)),
        "use_bv": bool(np.any(inputs["bv"] != 0)),
        "use_lnfw": not bool(np.all(inputs["ln_f_w"] == 1.0)),
        "use_lnfb": bool(np.any(inputs["ln_f_b"] != 0)),
        "use_lnw": not bool(np.all(inputs["ln_w"] == 1.0)),
        "use_lnb": bool(np.any(inputs["ln_b"] != 0)),
        "use_softplus": os.environ.get("KERNEL_SOFTPLUS", "") == "1",
    }
    cvals = {
        "c": float(inputs["scalar"][0]) ** 2 / 2.0,
        "b_order": float(inputs["b_order"][0]),
        "b_dist": float(inputs["b_dist"][0]),
    }
    return flags, cvals


def _shared_inputs(inputs, flags, cvals):
    hc = _host_constants()
    c = cvals["c"]
    Wq = inputs["Wq"].astype(np.float64)
    Wk = inputs["Wk"].astype(np.float64)
    wo, wd = inputs["W_order"].astype(np.float64), inputs["W_dist"].astype(np.float64)
    wblkq = np.zeros((H, 16), np.float64)
    wblkk = np.zeros((H, 16), np.float64)
    for h in range(NH):
        hs = slice(h * D, (h + 1) * D)
        wblkq[:, h] = Wq[:, hs] @ wo[:D, 0]
        wblkq[:, 8 + h] = Wq[:, hs] @ wd[:D, 0]
        wblkk[:, h] = Wk[:, hs] @ wo[D:, 0]
        wblkk[:, 8 + h] = Wk[:, hs] @ wd[D:, 0]
    cw = inputs["complex_weight"].astype(np.float32)
    shared = {
        "wq": inputs["Wq"].astype(np.float16),
        "wk": inputs["Wk"].astype(np.float16),
        "wv": inputs["Wv"].astype(np.float16),
        "wblkq": wblkq.astype(np.float16),
        "wblkk": wblkk.astype(np.float16),
        "lm": hc["Lm"], "lmt": hc["LmT"],
        "ones1": np.ones((1, S), np.float16),
        "dlA": np.vstack([hc["U1"].T, hc["P2"].T[0:20]]).astype(np.float16),
        "dlB": hc["P2"].T[20:24].astype(np.float16),
        "drA": (-c * hc["S2c"].T[0:20]).astype(np.float16),
        "drB": hc["V1"].T.astype(np.float16),
        "drC": (-c * hc["S2c"].T[20:24]).astype(np.float16),
        "cret": hc["cret"], "cimt": hc["cimt"],
        "irA": hc["irA"], "irB": hc["irB"],
        "wrt": np.ascontiguousarray(cw[0, :, :NF, 0].T).astype(np.float16),
        "wit": np.ascontiguousarray(cw[0, :, :NF, 1].T).astype(np.float16),
        "sel16": hc["sel16"],
    }
    if flags["use_bq"]:
        shared["bq"] = inputs["bq"].astype(np.float32)
    if flags["use_bk"]:
        shared["bk"] = inputs["bk"].astype(np.float32)
    if flags["use_bv"]:
        shared["bv"] = inputs["bv"].astype(np.float32)
    for nm, src in (("lnfw", "ln_f_w"), ("lnfb", "ln_f_b"),
                    ("lnw", "ln_w"), ("lnb", "ln_b")):
        if flags["use_" + nm]:
            shared[nm] = inputs[src].astype(np.float32)
    return shared


def kernel(**inputs):
    inputs = {k: np.asarray(v) for k, v in inputs.items()}
    x_all = inputs["input_tensor"].astype(np.float32)
    mask = inputs["attention_mask"].astype(np.float32)
    flags, cvals = _flags_cvals(inputs)

    key = (tuple(sorted(flags.items())), tuple(sorted(cvals.items())))
    if key not in _CACHE:
        _CACHE[key] = _build_program(cvals, flags)
    nc = _CACHE[key]

    shared = _shared_inputs(inputs, flags, cvals)
    in_maps = []
    for b in range(B):
        m = dict(shared)
        m["x"] = np.ascontiguousarray(x_all[b]).astype(np.float16)
        if flags["use_mask"]:
            m["m8"] = np.ascontiguousarray(8.0 * mask[b, 0, 0, :]).astype(np.float16)
        in_maps.append(m)

    import os
    trace = os.environ.get("KERNEL_TRACE", "") == "1"
    res = run_bass_kernel_spmd(nc, in_maps, core_ids=list(range(B)), trace=trace)
    global LAST_EXEC_NS, LAST_RESULTS
    LAST_RESULTS = res
    if res.exec_time_ns is not None:
        LAST_EXEC_NS = res.exec_time_ns
    out = np.stack([res.results[b]["out"] for b in range(B)]).astype(np.float32)
    return out


if __name__ == "__main__":
    print("kernel2 module ok")


# revision 4
# speedup vs baseline: 1.1051x; 1.0448x over previous
# Trainium2 Bass kernel for nn_Encoder_SelfAttention (sparse_attention), v2.
#
# Same contract as the baseline: kernel(**inputs) takes FULL unsharded inputs,
# shards batch across 8 cores, returns FULL (8,512,512) f32 output.
#
# v2 redesign (vs baseline at 144.5us):
# - Scores per (head, kt-block) built by ONE fused fp16 matmul with an
#   augmented K=112 contraction: rows 0..63 = q/k head rows (plain qk^T),
#   rows 64..87 = rank-24 SVD of -c*g^2, rows 88..111 = rank-12 SVD of g
#   paired with per-head 2c*dq / 2c*dk scaled basis rows. The tiny
#   -c*(dq+dk)^2 rank-1 terms (max 4e-3 in score units) are dropped.
#   PE matmul cost depends only on output columns, so folding all bias
#   terms into the contraction removes 4 of 5 score passes.
# - err_order: z = sign*(oq+ok) built by DVE/Pool stt into fp16; softplus as
#   a single ACT op (AF.Softplus, validated on hw; Exp+Ln fallback);
#   softplus SUBTRACTION done on PE via a -I fp16 matmul accumulated into
#   the scores PSUM group, so the final exp reads PSUM directly.
# - softmax denominator via ones-row in vaug (as baseline); reciprocal on
#   DVE (nc.vector.reciprocal), broadcast by a tiny PE matmul, and the
#   normalizing multiply on Pool (gpsimd) to keep DVE/ACT free.
# - FFT filter: fp16 DFT bases, Nyquist frequency dropped (validated
#   ~1e-3 end-to-end), filter products on DVE in fp16 (2x mode).
# - All big operands fp16 (weights, x for matmuls, sign matrix, bases):
#   halves DMA bytes; DMAs merged into one descriptor-batch per symbol.
# - z/softplus for ALL heads precomputed concurrently with projections so
#   the per-head PE stream (fused mm, -sp mm, ctx mm) never stalls on ACT.
import sys

sys.path.insert(0, "/opt/trn_rl_repo")

import numpy as np
from contextlib import ExitStack

import concourse.bass as bass
import concourse.tile as tile
from concourse import mybir
from concourse.bass_utils import run_bass_kernel_spmd
from concourse.masks import make_identity
from concourse.vector_clock import ScopedClock, VectorClock

F32 = mybir.dt.float32
F16 = mybir.dt.float16
R = mybir.dt.float32r
AF = mybir.ActivationFunctionType
ALU = mybir.AluOpType
B, S, H, NH, D = 8, 512, 512, 8, 64
NT = 4
R2, R1 = 24, 12          # SVD ranks for g^2 and g
KQ = 64                   # q/k head rows
KA = KQ + R2 + 2 * R1 + 3  # 115 fused contraction rows (116 with mask row)
NF = 256                  # kept rfft frequencies (Nyquist dropped)
# 32-aligned sub-blocks of the fused contraction (engine partition-start rule):
#   64..75  U (lhs, stt in0)      | V*2c*dq (rhs, stt out)
#   76..95  P[0:20]               | -c*S2[0:20]
#   96..107 U*2c*dk (lhs stt out) | V (rhs, stt in0)
#   108..111 P[20:24]             | -c*S2[20:24]
RU, RP0, RS, RP1 = 64, 76, 96, 108
# rows 112..114 (DMA-assembled, Taylor softplus): see _build_program
RT = 112


class _TileContext(tile.TileContext):
    # This walrus build rejects >1 sem wait on SP CTRL instructions; split
    # the tail-drain global-clock waits one-per-NOP. (Same as baseline.)
    def _drain_and_barrier(self, tick_clock, wait_clock):
        g = tick_clock.global_clock
        n = len(g)
        for i in range(n):
            if g[i] > 0:
                vec = [0] * n
                vec[i] = g[i]
                nop_inst = self.nc.sync.nop(nofuse=True)
                wait_clock.add_sem_waits(
                    nop_inst.ins, ScopedClock({None: VectorClock(vec)})
                )
        self.nc.sync.drain()
        self.nc.all_engine_barrier()
        assert self.sems is not None
        popped = self.nc._tile_sem_poison_stack.pop()
        assert popped is self._sem_poison
        self.nc.clear_and_free_semaphores(list(self.sems.allocated().values()))
        self.nc.all_engine_barrier()


def _split_excess_waits(nc):
    """Spill >cap sync-waits onto injected same-engine NOPs (walrus quirk)."""
    import bass_rust

    total = 0
    for fn in nc.m.functions:
        for blk in fn.blocks:
            out = []
            for inst in blk.instructions:
                si = inst.sync_info
                waits = list(si.on_wait) if si is not None else []
                cap = 2 if inst.__class__.__name__ == "InstEventSemaphore" else 1
                if len(waits) > cap:
                    keep, spill = waits[:cap], waits[cap:]
                    for w in spill:
                        nop = mybir.InstNoOp(
                            name=f"wsplit-{inst.name}-{total}", ins=[], outs=[])
                        nop.engine = inst.engine
                        nop.sync_info = bass_rust.SyncInfo(on_wait=[w], on_update=[])
                        out.append(nop)
                        total += 1
                    inst.sync_info = bass_rust.SyncInfo(
                        on_wait=keep, on_update=list(si.on_update))
                out.append(inst)
            blk.instructions = out
    return total


_HC = None


def _host_constants():
    """Input-independent structural constants (cached)."""
    global _HC
    if _HC is not None:
        return _HC
    idx = np.arange(S)
    g = np.log(np.abs(idx[None, :] - idx[:, None]).astype(np.float64) + 1.0)
    g2 = g ** 2
    u2, s2, vt2 = np.linalg.svd(g2)
    P2 = u2[:, :R2] * np.sqrt(s2[:R2])
    S2c = vt2[:R2].T * np.sqrt(s2[:R2])          # g2 ~= P2 @ S2c.T
    u1, s1, vt1 = np.linalg.svd(g)
    U1 = u1[:, :R1] * np.sqrt(s1[:R1])
    V1 = vt1[:R1].T * np.sqrt(s1[:R1])           # g ~= U1 @ V1.T
    # rfft/irfft ortho bases, Nyquist (freq 256) dropped
    W = np.fft.rfft(np.eye(H), norm="ortho", axis=-1)
    cret = np.ascontiguousarray(W.real[:, :NF]).astype(np.float16)   # [H, NF]
    cimt = np.ascontiguousarray(W.imag[:, :NF]).astype(np.float16)
    irA = np.fft.irfft(np.eye(257), n=H, norm="ortho", axis=-1)[:NF].astype(np.float16)
    irB = np.fft.irfft(1j * np.eye(257), n=H, norm="ortho", axis=-1)[:NF].astype(np.float16)
    Lm = np.where(idx[:, None] > idx[None, :], 0.5, -0.5).astype(np.float16)  # [k,q]
    sel16 = np.zeros((NH, NH * R1), np.float16)  # dk/dq head-row selector
    for h in range(NH):
        sel16[h, h * R1:(h + 1) * R1] = 1.0
    _HC = dict(g=g, g2=g2, P2=P2, S2c=S2c, U1=U1, V1=V1,
               cret=cret, cimt=cimt, irA=irA, irB=irB,
               Lm=Lm, LmT=np.ascontiguousarray(Lm.T), sel16=sel16)
    return _HC


def _build_program(c, flags):
    hc = _host_constants()
    nc = bass.Bass("TRN2", target_bir_lowering=False, debug=False)
    negc = -c["c"]
    twoc = 2.0 * c["c"]
    KF = KA + 1 if flags["use_mask"] else KA   # fused contraction depth

    def din(name, shape, dt):
        return nc.dram_tensor(name, list(shape), dt, kind="ExternalInput").ap()

    x_d = din("x", (S, H), F16)
    wq_d = din("wq", (H, H), F16)
    wk_d = din("wk", (H, H), F16)
    wv_d = din("wv", (H, H), F16)
    wblkq_d = din("wblkq", (H, 16), F16)
    wblkk_d = din("wblkk", (H, 16), F16)
    lm_d = din("lm", (S, S), F16)
    lmt_d = din("lmt", (S, S), F16)
    ones_d = din("ones1", (1, S), F16)
    dlA_d = din("dlA", (32, S), F16)         # [U1^T ; P2^T[0:20]]
    dlB_d = din("dlB", (4, S), F16)          # P2^T[20:24]
    drA_d = din("drA", (20, S), F16)         # -c*S2c^T[0:20]
    drB_d = din("drB", (R1, S), F16)         # V1^T
    drC_d = din("drC", (4, S), F16)          # -c*S2c^T[20:24]
    cret_d = din("cret", (H, NF), F16)
    cimt_d = din("cimt", (H, NF), F16)
    irA_d = din("irA", (NF, H), F16)
    irB_d = din("irB", (NF, H), F16)
    wrt_d = din("wrt", (NF, S), F16)
    wit_d = din("wit", (NF, S), F16)
    sel16_d = din("sel16", (NH, NH * R1), F16)
    if flags["use_mask"]:
        m8_d = din("m8", (S,), F16)
    if flags["use_bq"]:
        bq_d = din("bq", (H,), F32)
    if flags["use_bk"]:
        bk_d = din("bk", (H,), F32)
    if flags["use_bv"]:
        bv_d = din("bv", (H,), F32)
    ln_bcast = {}
    for nm in ("lnfw", "lnfb", "lnw", "lnb"):
        if flags["use_" + nm]:
            ln_bcast[nm] = din(nm, (H,), F32)
    out_d = nc.dram_tensor("out", [S, H], F32, kind="ExternalOutput").ap()
    import os
    dbg = os.environ.get("KERNEL_DEBUG", "") == "1"
    dbg_d = {}
    if dbg:
        for nm, shape, dt in (("d_xt0", (128, S), F16), ("d_lhs", (KF, NH * S), F16),
                              ("d_rhs", (KF, NH * S), F16),
                              ("d_et0", (128, NT * S), F32), ("d_cps0", (65, S), F32),
                              ("d_ctxt0", (128, S), F16), ("d_rows_oq", (8, S), F32),
                              ("d_okc0", (128, 8), F32)):
            dbg_d[nm] = nc.dram_tensor(nm, list(shape), dt, kind="ExternalOutput").ap()

    def blk_ap(d, rows, width, nblk, rep=False):
        """3D ap: HBM [rows*nblk, width] -> SBUF [rows, nblk*width].
        rep=True re-reads the same [rows,width] block nblk times."""
        return bass.AP(tensor=d.tensor, offset=0,
                       ap=[[width, rows], [0 if rep else rows * width, nblk],
                           [1, width]])

    def rep_load(engine, dst_tile_slice, d, rows, width, nblk):
        """Replicated load as nblk separate DMAs (no zero-stride free dim)."""
        ap0 = dst_tile_slice
        for b in range(nblk):
            sub = bass.AP(tensor=ap0.tensor, offset=ap0.offset + b * width,
                          ap=[list(ap0.ap[0]), [1, width]])
            engine.dma_start(sub, bass.AP(tensor=d.tensor, offset=0,
                                          ap=[[width, rows], [1, width]]))

    with _TileContext(nc) as tc:
        with ExitStack() as ctx:
            consts = ctx.enter_context(tc.tile_pool(name="consts", bufs=1))
            work = ctx.enter_context(tc.tile_pool(name="work", bufs=2))
            etp = ctx.enter_context(tc.tile_pool(name="etp", bufs=2))
            small = ctx.enter_context(tc.tile_pool(name="small", bufs=2))

            # ---- DMA loads (merged, ordered by first use) ----
            x16 = consts.tile([128, NT * S], F16, tag="x16")
            nc.sync.dma_start(x16[:, 0:2 * S],
                              bass.AP(tensor=x_d.tensor, offset=0,
                                      ap=[[S, 128], [128 * S, 2], [1, S]]))
            nc.sync.dma_start(x16[:, 2 * S:],
                              bass.AP(tensor=x_d.tensor, offset=2 * 128 * S,
                                      ap=[[S, 128], [128 * S, 2], [1, S]]))
            wq16 = consts.tile([128, NT * S], F16, tag="wq16")
            nc.scalar.dma_start(wq16[:], blk_ap(wq_d, 128, S, NT))
            wk16 = consts.tile([128, NT * S], F16, tag="wk16")
            nc.sync.dma_start(wk16[:], blk_ap(wk_d, 128, S, NT))
            wblkq16 = consts.tile([128, NT * 16], F16, tag="wblkq16")
            nc.scalar.dma_start(wblkq16[:], blk_ap(wblkq_d, 128, 16, NT))
            wblkk16 = consts.tile([128, NT * 16], F16, tag="wblkk16")
            nc.scalar.dma_start(wblkk16[:], blk_ap(wblkk_d, 128, 16, NT))
            lm_t = consts.tile([128, NT * S], F16, tag="lm")
            nc.sync.dma_start(lm_t[:], blk_ap(lm_d, 128, S, NT))
            lmt_t = consts.tile([128, NT * S], F16, tag="lmt")
            nc.sync.dma_start(lmt_t[:], blk_ap(lmt_d, 128, S, NT))
            sel16_t = consts.tile([NH, NH * R1], F16, tag="sel16")
            nc.sync.dma_start(sel16_t[:], sel16_d)

            # Fused-contraction operand tiles; aug rows replicated x8 by DMA
            lhs_all = consts.tile([KF, NH * S], F16, tag="lhs_all", name="lhs_all")
            rhs_all = consts.tile([KF, NH * S], F16, tag="rhs_all", name="rhs_all")
            nc.scalar.dma_start(lhs_all[RU:RU + 32, :], blk_ap(dlA_d, 32, S, NH, rep=True))
            nc.scalar.dma_start(lhs_all[RP1:RP1 + 4, :], blk_ap(dlB_d, 4, S, NH, rep=True))
            nc.sync.dma_start(rhs_all[RP0:RP0 + 20, :], blk_ap(drA_d, 20, S, NH, rep=True))
            nc.sync.dma_start(rhs_all[RS:RS + R1, :], blk_ap(drB_d, R1, S, NH, rep=True))
            nc.sync.dma_start(rhs_all[RP1:RP1 + 4, :], blk_ap(drC_d, 4, S, NH, rep=True))
            # Taylor rows 113(lhs)/114(rhs) are all-ones (host replicated)
            nc.scalar.dma_start(lhs_all[RT + 2:RT + 3, :],
                                blk_ap(ones_d, 1, S, NH, rep=True))
            nc.scalar.dma_start(rhs_all[RT + 1:RT + 2, :],
                                blk_ap(ones_d, 1, S, NH, rep=True))
            if flags["use_mask"]:
                nc.sync.dma_start(
                    lhs_all[KA:KA + 1, :],
                    bass.AP(tensor=m8_d.tensor, offset=0, ap=[[0, 1], [0, NH], [1, S]]))
                nc.scalar.dma_start(rhs_all[KA:KA + 1, :],
                                    blk_ap(ones_d, 1, S, NH, rep=True))

            wv16 = consts.tile([128, NT * S], F16, tag="wv16")
            nc.scalar.dma_start(wv16[:], blk_ap(wv_d, 128, S, NT))
            cret16 = consts.tile([128, NT * NF], F16, tag="cret16")
            nc.sync.dma_start(cret16[:], blk_ap(cret_d, 128, NF, NT))
            cimt16 = consts.tile([128, NT * NF], F16, tag="cimt16")
            nc.sync.dma_start(cimt16[:], blk_ap(cimt_d, 128, NF, NT))
            irA16 = consts.tile([128, 2 * S], F16, tag="irA16")
            nc.scalar.dma_start(irA16[:], blk_ap(irA_d, 128, S, 2))
            irB16 = consts.tile([128, 2 * S], F16, tag="irB16")
            nc.scalar.dma_start(irB16[:], blk_ap(irB_d, 128, S, 2))
            wrt16 = consts.tile([128, 2 * S], F16, tag="wrt16")
            nc.sync.dma_start(wrt16[:], blk_ap(wrt_d, 128, S, 2))
            wit16 = consts.tile([128, 2 * S], F16, tag="wit16")
            nc.sync.dma_start(wit16[:], blk_ap(wit_d, 128, S, 2))

            bias_cols = {}
            for nm, dd in (("bq", flags["use_bq"] and bq_d),
                           ("bk", flags["use_bk"] and bk_d)):
                if dd:
                    t = consts.tile([128, NT], F32, tag=nm)
                    nc.sync.dma_start(t[:], bass.AP(tensor=dd.tensor, offset=0,
                                                    ap=[[1, 128], [128, NT]]))
                    bias_cols[nm] = t
            if flags["use_bv"]:
                bv_row = consts.tile([1, H], F32, tag="bv")
                nc.sync.dma_start(bv_row[:], bass.AP(tensor=bv_d.tensor, offset=0,
                                                     ap=[[0, 1], [1, H]]))
            ln_bc = {}
            for nm, d_ap in ln_bcast.items():
                t = consts.tile([128, H], F32, tag=nm + "b")
                nc.gpsimd.dma_start(t[:], bass.AP(tensor=d_ap.tensor, offset=0,
                                                  ap=[[0, 128], [1, H]]))
                ln_bc[nm] = t

            # ---- small constants ----
            i16 = consts.tile([128, 128], F16, tag="i16")
            make_identity(nc, i16[:])
            i32 = consts.tile([8, 8], F32, tag="i32")
            make_identity(nc, i32[:])
            i32r = consts.tile([8, 8], F32, tag="i32r")
            nc.vector.tensor_copy(i32r[:].bitcast(R), i32[:])
            onescol0 = consts.tile([1, 128], F32, tag="onescol0")
            nc.vector.memset(onescol0[:], 1.0)
            onescol = consts.tile([1, 128], F32, tag="onescol")
            nc.vector.tensor_copy(onescol[:].bitcast(R), onescol0[:])
            ones_f = consts.tile([128, NH], F32, tag="ones_f")
            nc.vector.memset(ones_f[:], 1.0)
            _ccols = {}

            def constcol(val):
                if val not in _ccols:
                    t = consts.tile([128, 1], F32, tag=f"cc{len(_ccols)}")
                    nc.vector.memset(t[:], val)
                    _ccols[val] = t
                return _ccols[val]

            # ---- prologue: X^T ----
            xt16 = []
            diagp = ctx.enter_context(tc.tile_pool(name="diagp", bufs=6))
            with ExitStack() as pctx:
                pA = pctx.enter_context(tc.tile_pool(name="pA", bufs=1, space="PSUM"))
                pB = pctx.enter_context(tc.tile_pool(name="pB", bufs=5, space="PSUM"))
                for ht in range(NT):
                    tp = pA.tile([128, S], F16, tag="tp")
                    for st in range(NT):
                        nc.tensor.transpose(
                            tp[:, st * 128:(st + 1) * 128],
                            x16[:, st * S + ht * 128: st * S + (ht + 1) * 128],
                            i16[:])
                    t = consts.tile([128, S], F16, tag=f"xt{ht}", name=f"xt{ht}")
                    nc.vector.tensor_copy(t[:], tp[:])
                    if dbg and ht == 0:
                        nc.sync.dma_start(dbg_d["d_xt0"], t[:])
                    xt16.append(t)

                # ---- rows: oq/dq (q side), ok/dk (k side); separate base-0
                # PSUM groups per 8-row output ----
                def rows8(wblk, colbase, name, dt16, bias, eng):
                    psf = pB.tile([128, S], F32, tag="pj")
                    ps = psf[0:8, :]
                    for ht in range(NT):
                        nc.tensor.matmul(ps, wblk[:, ht * 16 + colbase:ht * 16 + colbase + 8],
                                         xt16[ht][:], start=(ht == 0),
                                         stop=(ht == NT - 1))
                    t = consts.tile([8, S], F16 if dt16 else F32, tag=name)
                    tout = t[:] if dt16 else t[:].bitcast(R)
                    if bias != 0.0:
                        nc.scalar.activation(tout, ps, AF.Identity,
                                             bias=constcol(float(bias))[0:8, 0:1],
                                             scale=1.0)
                    else:
                        eng.tensor_copy(tout, ps)
                    return t

                rows_oq = rows8(wblkq16[:], 0, "r_oq", False, c["b_order"], nc.vector)
                rows16_dq = rows8(wblkq16[:], 8, "r_dq", True, 0.0, nc.vector)
                rows_ok = rows8(wblkk16[:], 0, "r_ok", False, 0.0, nc.vector)
                rows16_dk = rows8(wblkk16[:], 8, "r_dk", True, c["b_dist"], nc.vector)

                # ok/oq columns [128,8] per block (diag-build scalar ptrs)
                okc, oqc = [], []
                for kt in range(NT):
                    ps = pB.tile([128, S], F32, tag="pj")
                    nc.tensor.matmul(ps[:, 0:8],
                                     rows_ok[:, kt * 128:(kt + 1) * 128].bitcast(R),
                                     i32r[:].bitcast(R), start=True, stop=False)
                    nc.tensor.matmul(ps[:, 8:16],
                                     rows_oq[:, kt * 128:(kt + 1) * 128].bitcast(R),
                                     i32r[:].bitcast(R), start=False, stop=True)
                    t = consts.tile([128, 16], F32, tag=f"okc{kt}")
                    nc.vector.tensor_copy(t[:], ps[:, 0:16])
                    okc.append(t)
                    oqc.append(t)

                # Taylor staging rows (all heads): w'=-ok^2/8, u'=-oq^2/8,
                # -oq/4, ok (fp16), then flat SBUF->SBUF DMAs scatter them
                # into aug rows 112..114 of each head block.
                ok16s = consts.tile([8, S], F16, tag="ok16s")
                nc.scalar.copy(ok16s[:], rows_ok[:])
                sqk = consts.tile([8, S], F16, tag="sqk")
                nc.scalar.square(sqk[:], rows_ok[:])
                w16s = consts.tile([8, S], F16, tag="w16s")
                nc.scalar.activation(w16s[:], sqk[:], AF.Identity, scale=-0.125)
                tq16s = consts.tile([8, S], F16, tag="tq16s")
                nc.scalar.activation(tq16s[:], rows_oq[:], AF.Identity, scale=-0.25)
                squ = consts.tile([8, S], F16, tag="squ")
                nc.scalar.square(squ[:], rows_oq[:])
                u16s = consts.tile([8, S], F16, tag="u16s")
                nc.scalar.activation(u16s[:], squ[:], AF.Identity, scale=-0.125)

                def flat_row(dst_row, srct):
                    nc.sync.dma_start(dst_row, srct[:])

                flat_row(lhs_all[RT:RT + 1, :], ok16s)       # ok  | -oq/4
                flat_row(rhs_all[RT:RT + 1, :], tq16s)
                flat_row(lhs_all[RT + 1:RT + 2, :], w16s)    # w'  | ones
                flat_row(rhs_all[RT + 2:RT + 3, :], u16s)    # ones| u' 

                # ---- per-head scaled SVD basis rows ----
                for h in range(NH):
                    hsl = slice(h * S, (h + 1) * S)
                    dkb = pB.tile([128, S], F32, tag="pj")
                    nc.tensor.matmul(dkb[0:R1, :], sel16_t[:, h * R1:(h + 1) * R1],
                                     rows16_dk[:], start=True, stop=True)
                    nc.vector.scalar_tensor_tensor(
                        lhs_all[RS:RS + R1, hsl],
                        lhs_all[RU:RU + R1, hsl], twoc, dkb[0:R1, :],
                        op0=ALU.mult, op1=ALU.mult)
                    dqb = pB.tile([128, S], F32, tag="pj")
                    nc.tensor.matmul(dqb[0:R1, :], sel16_t[:, h * R1:(h + 1) * R1],
                                     rows16_dq[:], start=True, stop=True)
                    nc.vector.scalar_tensor_tensor(
                        rhs_all[RU:RU + R1, hsl],
                        rhs_all[RS:RS + R1, hsl], twoc, dqb[0:R1, :],
                        op0=ALU.mult, op1=ALU.mult)

                # ---- projections: q/k head rows into rhs_all/lhs_all ----
                for ot in range(NT):
                    psq = pB.tile([128, S], F32, tag="pj")
                    for ht in range(NT):
                        nc.tensor.matmul(psq[:], wq16[:, ht * S + ot * 128:ht * S + (ot + 1) * 128],
                                         xt16[ht][:], start=(ht == 0), stop=(ht == NT - 1))
                    psk = pB.tile([128, S], F32, tag="pj")
                    for ht in range(NT):
                        nc.tensor.matmul(psk[:], wk16[:, ht * S + ot * 128:ht * S + (ot + 1) * 128],
                                         xt16[ht][:], start=(ht == 0), stop=(ht == NT - 1))
                    for po, h in ((0, 2 * ot), (64, 2 * ot + 1)):
                        hsl = slice(h * S, (h + 1) * S)
                        if flags["use_bq"]:
                            nc.scalar.activation(rhs_all[0:KQ, hsl], psq[po:po + 64, :],
                                                 AF.Identity,
                                                 bias=bias_cols["bq"][po:po + 64, ot:ot + 1],
                                                 scale=1.0)
                        else:
                            nc.scalar.activation(rhs_all[0:KQ, hsl], psq[po:po + 64, :],
                                                 AF.Identity)
                        if flags["use_bk"]:
                            nc.scalar.activation(lhs_all[0:KQ, hsl], psk[po:po + 64, :],
                                                 AF.Identity,
                                                 bias=bias_cols["bk"][po:po + 64, ot:ot + 1],
                                                 scale=1.0)
                        else:
                            nc.vector.tensor_copy(lhs_all[0:KQ, hsl], psk[po:po + 64, :])

                # ---- V projection -> vaug (value rows + ones column) ----
                vaug = []
                for st in range(NT):
                    ps = pB.tile([128, S], F32, tag="pj")
                    for ht in range(NT):
                        nc.tensor.matmul(ps[:], xt16[ht][:, st * 128:(st + 1) * 128],
                                         wv16[:, ht * S:(ht + 1) * S],
                                         start=(ht == 0),
                                         stop=(ht == NT - 1 and not flags["use_bv"]))
                    if flags["use_bv"]:
                        nc.tensor.matmul(ps[:], onescol[:].bitcast(R),
                                         bv_row[:].bitcast(R), start=False, stop=True)
                    t = consts.tile([128, NH * 65], F32, tag=f"vaug{st}", name=f"vaug{st}")
                    tap = t[:]
                    ones_cols = bass.AP(tensor=tap.tensor, offset=tap.offset + D,
                                        ap=[list(tap.ap[0]), [65, NH], [1, 1]])
                    nc.vector.tensor_copy(ones_cols.bitcast(R), ones_f[:])
                    dst = bass.AP(tensor=tap.tensor, offset=tap.offset,
                                  ap=[list(tap.ap[0]), [65, NH], [1, D]])
                    nc.vector.tensor_copy(dst.bitcast(R), ps[:])
                    vaug.append(t)
            # ---- head loop ----
            ctxt16 = [consts.tile([128, S], F16, tag=f"ctxt{ht}", name=f"ctxt{ht}")
                      for ht in range(NT)]
            if dbg:
                nc.sync.dma_start(dbg_d["d_lhs"], lhs_all[0:KF, :])
                nc.sync.dma_start(dbg_d["d_rhs"], rhs_all[0:KF, :])
                nc.sync.dma_start(dbg_d["d_rows_oq"], rows_oq[:])
                nc.sync.dma_start(dbg_d["d_okc0"], okc[0][:])
            rfp = ctx.enter_context(tc.tile_pool(name="rfp", bufs=1, space="PSUM"))
            rt0_ps = rfp.tile([128, S], F32, tag="rt0")
            it0_ps = rfp.tile([128, S], F32, tag="it0")
            with ExitStack() as lctx:
                scp = lctx.enter_context(
                    tc.tile_pool(name="scp", bufs=2, space="PSUM"))
                ctxp = lctx.enter_context(
                    tc.tile_pool(name="ctxp", bufs=1, space="PSUM"))
                rbpp = lctx.enter_context(
                    tc.tile_pool(name="rbpp", bufs=1, space="PSUM"))
                for h in range(NH):
                    hb = h * S
                    et = etp.tile([128, NT * S], R, tag="et")
                    cps = ctxp.tile([65, S], F32, tag="cps")

                    def ctx_mm(kt):
                        nc.tensor.matmul(cps[:], vaug[kt][:, h * 65:(h + 1) * 65].bitcast(R),
                                         et[:, kt * S:(kt + 1) * S],
                                         start=(kt == 0), stop=(kt == NT - 1))

                    doq = []
                    for qt in range(NT):
                        dt_ = diagp.tile([128, 128], F16, tag="doq")
                        nc.gpsimd.tensor_scalar_mul(dt_[:], i16[:],
                                                    oqc[qt][:, 8 + h:9 + h])
                        doq.append(dt_)
                    for half in range(2):
                        o = scp.tile([128, 2 * S], F32, tag="sc")
                        for kt in (2 * half, 2 * half + 1):
                            osl = o[:, (kt % 2) * S:(kt % 2) * S + S]
                            nc.tensor.matmul(osl, lhs_all[0:KF, hb + kt * 128:hb + (kt + 1) * 128],
                                             rhs_all[0:KF, hb:hb + S], start=True, stop=False)
                            dok = diagp.tile([128, 128], F16, tag="dok")
                            nc.gpsimd.tensor_scalar_mul(dok[:], i16[:],
                                                        okc[kt][:, h:h + 1])
                            nc.tensor.matmul(osl, dok[:], lm_t[:, kt * S:(kt + 1) * S],
                                             start=False, stop=False)
                            for qt in range(NT):
                                nc.tensor.matmul(
                                    osl[:, qt * 128:(qt + 1) * 128],
                                    lmt_t[:, qt * S + kt * 128:qt * S + (kt + 1) * 128],
                                    doq[qt][:], start=False, stop=(qt == NT - 1))
                        nc.scalar.activation(et[:, half * 2 * S:(half + 1) * 2 * S], o[:],
                                             AF.Exp, scale=0.125)
                        if half == 1:
                            ctx_mm(0)
                            ctx_mm(1)
                    ctx_mm(2)
                    ctx_mm(3)
                    # normalization: reciprocal of the denom row, broadcast via
                    # PE, fp16 copy + multiply on Pool
                    if dbg and h == 0:
                        etsb = work.tile([128, NT * S], F32, tag="dbget")
                        nc.vector.tensor_copy(etsb[:], et[:])
                        nc.sync.dma_start(dbg_d["d_et0"], etsb[:])
                        cpsb = work.tile([65, S], F32, tag="dbgcps")
                        nc.vector.tensor_copy(cpsb[:], cps[:])
                        nc.sync.dma_start(dbg_d["d_cps0"], cpsb[:])
                    rc = small.tile([1, S], mybir.dt.float32r, tag="rc")
                    with nc.allow_low_precision(reason="softmax denom reciprocal to f32r"):
                        nc.vector.reciprocal(rc[:], cps[64:65, :])
                    rbp = rbpp.tile([64, S], F32, tag="rbp")
                    nc.tensor.matmul(rbp[:], onescol[0:1, 0:64].bitcast(R), rc[:],
                                     start=True, stop=True)
                    rbs = work.tile([64, S], F16, tag="rbs")
                    nc.vector.tensor_copy(rbs[:], rbp[:])
                    po = (h % 2) * 64
                    nc.vector.tensor_tensor(ctxt16[h // 2][po:po + 64, :],
                                            cps[0:64, :], rbs[:], ALU.mult)
                    if h % 2 == 1:
                        ht = h // 2
                        nc.tensor.matmul(rt0_ps[:], cret16[:, ht * NF:ht * NF + 128],
                                         ctxt16[ht][:], start=(ht == 0), stop=(ht == NT - 1))
                        nc.tensor.matmul(it0_ps[:], cimt16[:, ht * NF:ht * NF + 128],
                                         ctxt16[ht][:], start=(ht == 0), stop=(ht == NT - 1))

            if dbg:
                nc.sync.dma_start(dbg_d["d_ctxt0"], ctxt16[0][:])
            # ---- FFT filter + residual + layernorms (tail) ----
            with ExitStack() as fctx:
                fftp = fctx.enter_context(
                    tc.tile_pool(name="fftp", bufs=1, space="PSUM"))
                miscp = fctx.enter_context(
                    tc.tile_pool(name="miscp", bufs=2, space="PSUM"))
                pr16, pi16 = [], []
                for ft in range(2):
                    if ft == 0:
                        rt_ps, it_ps = rt0_ps, it0_ps
                    else:
                        rt_ps = fftp.tile([128, S], F32, tag=f"rt{ft}")
                        it_ps = fftp.tile([128, S], F32, tag=f"it{ft}")
                        for ht in range(NT):
                            nc.tensor.matmul(rt_ps[:], cret16[:, ht * NF + ft * 128:ht * NF + (ft + 1) * 128],
                                             ctxt16[ht][:], start=(ht == 0), stop=(ht == NT - 1))
                        for ht in range(NT):
                            nc.tensor.matmul(it_ps[:], cimt16[:, ht * NF + ft * 128:ht * NF + (ft + 1) * 128],
                                             ctxt16[ht][:], start=(ht == 0), stop=(ht == NT - 1))
                    rts = work.tile([128, S], F16, tag="rts")
                    nc.vector.tensor_copy(rts[:], rt_ps[:])
                    its = work.tile([128, S], F16, tag="its")
                    nc.vector.tensor_copy(its[:], it_ps[:])
                    wrs = wrt16[:, ft * S:(ft + 1) * S]
                    wis = wit16[:, ft * S:(ft + 1) * S]
                    t1 = work.tile([128, S], F16, tag="f1")
                    t2 = work.tile([128, S], F16, tag="f2")
                    nc.vector.tensor_tensor(t1[:], rts[:], wrs, ALU.mult)
                    nc.vector.tensor_tensor(t2[:], its[:], wis, ALU.mult)
                    pr = consts.tile([128, S], F16, tag=f"pr{ft}", name=f"pr{ft}")
                    nc.vector.tensor_tensor(pr[:], t1[:], t2[:], ALU.subtract)
                    pr16.append(pr)
                    nc.vector.tensor_tensor(t1[:], rts[:], wis, ALU.mult)
                    nc.vector.tensor_tensor(t2[:], its[:], wrs, ALU.mult)
                    pi = consts.tile([128, S], F16, tag=f"pi{ft}", name=f"pi{ft}")
                    nc.vector.tensor_tensor(pi[:], t1[:], t2[:], ALU.add)
                    pi16.append(pi)

                def layer_norm(dst, src, wname, bname, tagn):
                    st6 = small.tile([128, 6], F32, tag="st6" + tagn)
                    nc.vector.bn_stats(st6[:], src)
                    mv = small.tile([128, 2], F32, tag="mv" + tagn)
                    nc.vector.bn_aggr(mv[:], st6[:])
                    lnv = small.tile([128, 1], F32, tag="lnv" + tagn)
                    nc.scalar.activation(lnv[:], mv[:, 1:2], AF.Ln,
                                         bias=constcol(1e-12)[:, 0:1], scale=1.0)
                    rs = small.tile([128, 1], F32, tag="rs" + tagn)
                    nc.scalar.activation(rs[:], lnv[:], AF.Exp, scale=-0.5)
                    nb = small.tile([128, 1], F32, tag="nb" + tagn)
                    nc.vector.scalar_tensor_tensor(
                        nb[:], mv[:, 0:1], -1.0, rs[:],
                        op0=ALU.mult, op1=ALU.mult)
                    nc.scalar.activation(dst, src, AF.Identity,
                                         bias=nb[:, 0:1], scale=rs[:, 0:1])
                    if flags["use_" + wname]:
                        nc.vector.tensor_mul(dst, dst, ln_bc[wname][:])
                    if flags["use_" + bname]:
                        nc.vector.tensor_add(dst, dst, ln_bc[bname][:])

                for st in range(NT):
                    ssl = slice(st * 128, (st + 1) * 128)
                    yp = miscp.tile([128, S], F32, tag="yp")
                    for ft in range(2):
                        nc.tensor.matmul(yp[:], pr16[ft][:, ssl],
                                         irA16[:, ft * S:(ft + 1) * S],
                                         start=(ft == 0), stop=False)
                        nc.tensor.matmul(yp[:], pi16[ft][:, ssl],
                                         irB16[:, ft * S:(ft + 1) * S],
                                         start=False, stop=False)
                    for ht in range(NT):
                        nc.tensor.matmul(yp[:, ht * 128:(ht + 1) * 128],
                                         ctxt16[ht][:, ssl], i16[:],
                                         start=False, stop=(ht == NT - 1))
                    hid = work.tile([128, S], F32, tag="hid")
                    layer_norm(hid[:], yp[:], "lnfw", "lnfb", "a")
                    r2t = work.tile([128, S], F32, tag="r2")
                    nc.gpsimd.tensor_add(r2t[:], hid[:], x16[:, st * S:(st + 1) * S])
                    osb = work.tile([128, S], F32, tag="osb")
                    layer_norm(osb[:], r2t[:], "lnw", "lnb", "b")
                    nc.sync.dma_start(out_d[ssl, :], osb[:])

    nsplit = _split_excess_waits(nc)
    if nsplit:
        print(f"[kernel2] split {nsplit} excess sync waits onto NOPs")
    return nc


_CACHE = {}
LAST_EXEC_NS = None
LAST_RESULTS = None


def _flags_cvals(inputs):
    import os
    flags = {
        "use_mask": bool(np.any(inputs["attention_mask"] != 0)),
        "use_bq": bool(np.any(inputs["bq"] != 0)),
        "use_bk": bool(np.any(inputs["bk"] != 0)),
        "use_bv": bool(np.any(inputs["bv"] != 0)),
        "use_lnfw": not bool(np.all(inputs["ln_f_w"] == 1.0)),
        "use_lnfb": bool(np.any(inputs["ln_f_b"] != 0)),
        "use_lnw": not bool(np.all(inputs["ln_w"] == 1.0)),
        "use_lnb": bool(np.any(inputs["ln_b"] != 0)),
        "use_softplus": os.environ.get("KERNEL_SOFTPLUS", "") == "1",
    }
    cvals = {
        "c": float(inputs["scalar"][0]) ** 2 / 2.0,
        "b_order": float(inputs["b_order"][0]),
        "b_dist": float(inputs["b_dist"][0]),
    }
    return flags, cvals


def _shared_inputs(inputs, flags, cvals):
    hc = _host_constants()
    c = cvals["c"]
    Wq = inputs["Wq"].astype(np.float64)
    Wk = inputs["Wk"].astype(np.float64)
    wo, wd = inputs["W_order"].astype(np.float64), inputs["W_dist"].astype(np.float64)
    wblkq = np.zeros((H, 16), np.float64)
    wblkk = np.zeros((H, 16), np.float64)
    for h in range(NH):
        hs = slice(h * D, (h + 1) * D)
        wblkq[:, h] = Wq[:, hs] @ wo[:D, 0]
        wblkq[:, 8 + h] = Wq[:, hs] @ wd[:D, 0]
        wblkk[:, h] = Wk[:, hs] @ wo[D:, 0]
        wblkk[:, 8 + h] = Wk[:, hs] @ wd[D:, 0]
    cw = inputs["complex_weight"].astype(np.float32)
    shared = {
        "wq": inputs["Wq"].astype(np.float16),
        "wk": inputs["Wk"].astype(np.float16),
        "wv": inputs["Wv"].astype(np.float16),
        "wblkq": wblkq.astype(np.float16),
        "wblkk": wblkk.astype(np.float16),
        "lm": hc["Lm"], "lmt": hc["LmT"],
        "ones1": np.ones((1, S), np.float16),
        "dlA": np.vstack([hc["U1"].T, hc["P2"].T[0:20]]).astype(np.float16),
        "dlB": hc["P2"].T[20:24].astype(np.float16),
        "drA": (-c * hc["S2c"].T[0:20]).astype(np.float16),
        "drB": hc["V1"].T.astype(np.float16),
        "drC": (-c * hc["S2c"].T[20:24]).astype(np.float16),
        "cret": hc["cret"], "cimt": hc["cimt"],
        "irA": hc["irA"], "irB": hc["irB"],
        "wrt": np.ascontiguousarray(cw[0, :, :NF, 0].T).astype(np.float16),
        "wit": np.ascontiguousarray(cw[0, :, :NF, 1].T).astype(np.float16),
        "sel16": hc["sel16"],
    }
    if flags["use_bq"]:
        shared["bq"] = inputs["bq"].astype(np.float32)
    if flags["use_bk"]:
        shared["bk"] = inputs["bk"].astype(np.float32)
    if flags["use_bv"]:
        shared["bv"] = inputs["bv"].astype(np.float32)
    for nm, src in (("lnfw", "ln_f_w"), ("lnfb", "ln_f_b"),
                    ("lnw", "ln_w"), ("lnb", "ln_b")):
        if flags["use_" + nm]:
            shared[nm] = inputs[src].astype(np.float32)
    return shared


def kernel(**inputs):
    inputs = {k: np.asarray(v) for k, v in inputs.items()}
    x_all = inputs["input_tensor"].astype(np.float32)
    mask = inputs["attention_mask"].astype(np.float32)
    flags, cvals = _flags_cvals(inputs)

    key = (tuple(sorted(flags.items())), tuple(sorted(cvals.items())))
    if key not in _CACHE:
        _CACHE[key] = _build_program(cvals, flags)
    nc = _CACHE[key]

    shared = _shared_inputs(inputs, flags, cvals)
    in_maps = []
    for b in range(B):
        m = dict(shared)
        m["x"] = np.ascontiguousarray(x_all[b]).astype(np.float16)
        if flags["use_mask"]:
            m["m8"] = np.ascontiguousarray(8.0 * mask[b, 0, 0, :]).astype(np.float16)
        in_maps.append(m)

    import os
    trace = os.environ.get("KERNEL_TRACE", "") == "1"
    res = run_bass_kernel_spmd(nc, in_maps, core_ids=list(range(B)), trace=trace)
    global LAST_EXEC_NS, LAST_RESULTS
    LAST_RESULTS = res
    if res.exec_time_ns is not None:
        LAST_EXEC_NS = res.exec_time_ns
    out = np.stack([res.results[b]["out"] for b in range(B)]).astype(np.float32)
    return out


if __name__ == "__main__":
    print("kernel2 module ok")


# revision 5
# speedup vs baseline: 1.1177x; 1.0113x over previous
# Trainium2 Bass kernel for nn_Encoder_SelfAttention (sparse_attention), v2.
#
# Same contract as the baseline: kernel(**inputs) takes FULL unsharded inputs,
# shards batch across 8 cores, returns FULL (8,512,512) f32 output.
#
# v2 redesign (vs baseline at 144.5us):
# - Scores per (head, kt-block) built by ONE fused fp16 matmul with an
#   augmented K=112 contraction: rows 0..63 = q/k head rows (plain qk^T),
#   rows 64..87 = rank-24 SVD of -c*g^2, rows 88..111 = rank-12 SVD of g
#   paired with per-head 2c*dq / 2c*dk scaled basis rows. The tiny
#   -c*(dq+dk)^2 rank-1 terms (max 4e-3 in score units) are dropped.
#   PE matmul cost depends only on output columns, so folding all bias
#   terms into the contraction removes 4 of 5 score passes.
# - err_order: z = sign*(oq+ok) built by DVE/Pool stt into fp16; softplus as
#   a single ACT op (AF.Softplus, validated on hw; Exp+Ln fallback);
#   softplus SUBTRACTION done on PE via a -I fp16 matmul accumulated into
#   the scores PSUM group, so the final exp reads PSUM directly.
# - softmax denominator via ones-row in vaug (as baseline); reciprocal on
#   DVE (nc.vector.reciprocal), broadcast by a tiny PE matmul, and the
#   normalizing multiply on Pool (gpsimd) to keep DVE/ACT free.
# - FFT filter: fp16 DFT bases, Nyquist frequency dropped (validated
#   ~1e-3 end-to-end), filter products on DVE in fp16 (2x mode).
# - All big operands fp16 (weights, x for matmuls, sign matrix, bases):
#   halves DMA bytes; DMAs merged into one descriptor-batch per symbol.
# - z/softplus for ALL heads precomputed concurrently with projections so
#   the per-head PE stream (fused mm, -sp mm, ctx mm) never stalls on ACT.
import sys

sys.path.insert(0, "/opt/trn_rl_repo")

import numpy as np
from contextlib import ExitStack

import concourse.bass as bass
import concourse.tile as tile
from concourse import mybir
from concourse.bass_utils import run_bass_kernel_spmd
from concourse.masks import make_identity
from concourse.vector_clock import ScopedClock, VectorClock

F32 = mybir.dt.float32
F16 = mybir.dt.float16
R = mybir.dt.float32r
AF = mybir.ActivationFunctionType
ALU = mybir.AluOpType
B, S, H, NH, D = 8, 512, 512, 8, 64
NT = 4
R2, R1 = 24, 12          # SVD ranks for g^2 and g
KQ = 64                   # q/k head rows
KA = KQ + R2 + 2 * R1 + 3  # 115 fused contraction rows (116 with mask row)
NF = 256                  # kept rfft frequencies (Nyquist dropped)
# 32-aligned sub-blocks of the fused contraction (engine partition-start rule):
#   64..75  U (lhs, stt in0)      | V*2c*dq (rhs, stt out)
#   76..95  P[0:20]               | -c*S2[0:20]
#   96..107 U*2c*dk (lhs stt out) | V (rhs, stt in0)
#   108..111 P[20:24]             | -c*S2[20:24]
RU, RP0, RS, RP1 = 64, 76, 96, 108
# rows 112..114 (DMA-assembled, Taylor softplus): see _build_program
RT = 112


class _TileContext(tile.TileContext):
    # This walrus build rejects >1 sem wait on SP CTRL instructions; split
    # the tail-drain global-clock waits one-per-NOP. (Same as baseline.)
    def _drain_and_barrier(self, tick_clock, wait_clock):
        g = tick_clock.global_clock
        n = len(g)
        for i in range(n):
            if g[i] > 0:
                vec = [0] * n
                vec[i] = g[i]
                nop_inst = self.nc.sync.nop(nofuse=True)
                wait_clock.add_sem_waits(
                    nop_inst.ins, ScopedClock({None: VectorClock(vec)})
                )
        self.nc.sync.drain()
        self.nc.all_engine_barrier()
        assert self.sems is not None
        popped = self.nc._tile_sem_poison_stack.pop()
        assert popped is self._sem_poison
        self.nc.clear_and_free_semaphores(list(self.sems.allocated().values()))
        self.nc.all_engine_barrier()


def _split_excess_waits(nc):
    """Spill >cap sync-waits onto injected same-engine NOPs (walrus quirk)."""
    import bass_rust

    total = 0
    for fn in nc.m.functions:
        for blk in fn.blocks:
            out = []
            for inst in blk.instructions:
                si = inst.sync_info
                waits = list(si.on_wait) if si is not None else []
                cap = 2 if inst.__class__.__name__ == "InstEventSemaphore" else 1
                if len(waits) > cap:
                    keep, spill = waits[:cap], waits[cap:]
                    for w in spill:
                        nop = mybir.InstNoOp(
                            name=f"wsplit-{inst.name}-{total}", ins=[], outs=[])
                        nop.engine = inst.engine
                        nop.sync_info = bass_rust.SyncInfo(on_wait=[w], on_update=[])
                        out.append(nop)
                        total += 1
                    inst.sync_info = bass_rust.SyncInfo(
                        on_wait=keep, on_update=list(si.on_update))
                out.append(inst)
            blk.instructions = out
    return total


_HC = None


def _host_constants():
    """Input-independent structural constants (cached)."""
    global _HC
    if _HC is not None:
        return _HC
    idx = np.arange(S)
    g = np.log(np.abs(idx[None, :] - idx[:, None]).astype(np.float64) + 1.0)
    g2 = g ** 2
    u2, s2, vt2 = np.linalg.svd(g2)
    P2 = u2[:, :R2] * np.sqrt(s2[:R2])
    S2c = vt2[:R2].T * np.sqrt(s2[:R2])          # g2 ~= P2 @ S2c.T
    u1, s1, vt1 = np.linalg.svd(g)
    U1 = u1[:, :R1] * np.sqrt(s1[:R1])
    V1 = vt1[:R1].T * np.sqrt(s1[:R1])           # g ~= U1 @ V1.T
    # rfft/irfft ortho bases, Nyquist (freq 256) dropped
    W = np.fft.rfft(np.eye(H), norm="ortho", axis=-1)
    cret = np.ascontiguousarray(W.real[:, :NF]).astype(np.float16)   # [H, NF]
    cimt = np.ascontiguousarray(W.imag[:, :NF]).astype(np.float16)
    irA = np.fft.irfft(np.eye(257), n=H, norm="ortho", axis=-1)[:NF].astype(np.float16)
    irB = np.fft.irfft(1j * np.eye(257), n=H, norm="ortho", axis=-1)[:NF].astype(np.float16)
    Lm = np.where(idx[:, None] > idx[None, :], 0.5, -0.5).astype(np.float16)  # [k,q]
    sel16 = np.zeros((NH, NH * R1), np.float16)  # dk/dq head-row selector
    for h in range(NH):
        sel16[h, h * R1:(h + 1) * R1] = 1.0
    _HC = dict(g=g, g2=g2, P2=P2, S2c=S2c, U1=U1, V1=V1,
               cret=cret, cimt=cimt, irA=irA, irB=irB,
               Lm=Lm, LmT=np.ascontiguousarray(Lm.T), sel16=sel16)
    return _HC


def _build_program(c, flags):
    hc = _host_constants()
    nc = bass.Bass("TRN2", target_bir_lowering=False, debug=False)
    negc = -c["c"]
    twoc = 2.0 * c["c"]
    KF = KA + 1 if flags["use_mask"] else KA   # fused contraction depth

    def din(name, shape, dt):
        return nc.dram_tensor(name, list(shape), dt, kind="ExternalInput").ap()

    x_d = din("x", (S, H), F16)
    wq_d = din("wq", (H, H), F16)
    wk_d = din("wk", (H, H), F16)
    wv_d = din("wv", (H, H), F16)
    wblkq_d = din("wblkq", (H, 16), F16)
    wblkk_d = din("wblkk", (H, 16), F16)
    lm_d = din("lm", (S, S), F16)
    lmt_d = din("lmt", (S, S), F16)
    ones_d = din("ones1", (1, S), F16)
    dlA_d = din("dlA", (32, S), F16)         # [U1^T ; P2^T[0:20]]
    dlB_d = din("dlB", (4, S), F16)          # P2^T[20:24]
    drA_d = din("drA", (20, S), F16)         # -c*S2c^T[0:20]
    drB_d = din("drB", (R1, S), F16)         # V1^T
    drC_d = din("drC", (4, S), F16)          # -c*S2c^T[20:24]
    cret_d = din("cret", (H, NF), F16)
    cimt_d = din("cimt", (H, NF), F16)
    irA_d = din("irA", (NF, H), F16)
    irB_d = din("irB", (NF, H), F16)
    wrt_d = din("wrt", (NF, S), F16)
    wit_d = din("wit", (NF, S), F16)
    sel16_d = din("sel16", (NH, NH * R1), F16)
    if flags["use_mask"]:
        m8_d = din("m8", (S,), F16)
    if flags["use_bq"]:
        bq_d = din("bq", (H,), F32)
    if flags["use_bk"]:
        bk_d = din("bk", (H,), F32)
    if flags["use_bv"]:
        bv_d = din("bv", (H,), F32)
    ln_bcast = {}
    for nm in ("lnfw", "lnfb", "lnw", "lnb"):
        if flags["use_" + nm]:
            ln_bcast[nm] = din(nm, (H,), F32)
    out_d = nc.dram_tensor("out", [S, H], F32, kind="ExternalOutput").ap()
    import os
    dbg = os.environ.get("KERNEL_DEBUG", "") == "1"
    dbg_d = {}
    if dbg:
        for nm, shape, dt in (("d_xt0", (128, S), F16), ("d_lhs", (KF, NH * S), F16),
                              ("d_rhs", (KF, NH * S), F16),
                              ("d_et0", (128, NT * S), F32), ("d_cps0", (65, S), F32),
                              ("d_ctxt0", (128, S), F16), ("d_rows_oq", (8, S), F32),
                              ("d_okc0", (128, 8), F32)):
            dbg_d[nm] = nc.dram_tensor(nm, list(shape), dt, kind="ExternalOutput").ap()

    def blk_ap(d, rows, width, nblk, rep=False):
        """3D ap: HBM [rows*nblk, width] -> SBUF [rows, nblk*width].
        rep=True re-reads the same [rows,width] block nblk times."""
        return bass.AP(tensor=d.tensor, offset=0,
                       ap=[[width, rows], [0 if rep else rows * width, nblk],
                           [1, width]])

    def rep_load(engine, dst_tile_slice, d, rows, width, nblk):
        """Replicated load as nblk separate DMAs (no zero-stride free dim)."""
        ap0 = dst_tile_slice
        for b in range(nblk):
            sub = bass.AP(tensor=ap0.tensor, offset=ap0.offset + b * width,
                          ap=[list(ap0.ap[0]), [1, width]])
            engine.dma_start(sub, bass.AP(tensor=d.tensor, offset=0,
                                          ap=[[width, rows], [1, width]]))

    with _TileContext(nc) as tc:
        with ExitStack() as ctx:
            consts = ctx.enter_context(tc.tile_pool(name="consts", bufs=1))
            work = ctx.enter_context(tc.tile_pool(name="work", bufs=2))
            etp = ctx.enter_context(tc.tile_pool(name="etp", bufs=2))
            small = ctx.enter_context(tc.tile_pool(name="small", bufs=2))

            # ---- DMA loads (merged, ordered by first use) ----
            x16 = consts.tile([128, NT * S], F16, tag="x16")
            nc.sync.dma_start(x16[:, 0:2 * S],
                              bass.AP(tensor=x_d.tensor, offset=0,
                                      ap=[[S, 128], [128 * S, 2], [1, S]]))
            nc.sync.dma_start(x16[:, 2 * S:],
                              bass.AP(tensor=x_d.tensor, offset=2 * 128 * S,
                                      ap=[[S, 128], [128 * S, 2], [1, S]]))
            wq16 = consts.tile([128, NT * S], F16, tag="wq16")
            nc.scalar.dma_start(wq16[:], blk_ap(wq_d, 128, S, NT))
            wk16 = consts.tile([128, NT * S], F16, tag="wk16")
            nc.sync.dma_start(wk16[:], blk_ap(wk_d, 128, S, NT))
            wblkq16 = consts.tile([128, NT * 16], F16, tag="wblkq16")
            nc.scalar.dma_start(wblkq16[:], blk_ap(wblkq_d, 128, 16, NT))
            wblkk16 = consts.tile([128, NT * 16], F16, tag="wblkk16")
            nc.scalar.dma_start(wblkk16[:], blk_ap(wblkk_d, 128, 16, NT))
            lm_t = consts.tile([128, NT * S], F16, tag="lm")
            nc.sync.dma_start(lm_t[:], blk_ap(lm_d, 128, S, NT))
            lmt_t = consts.tile([128, NT * S], F16, tag="lmt")
            nc.sync.dma_start(lmt_t[:], blk_ap(lmt_d, 128, S, NT))
            sel16_t = consts.tile([NH, NH * R1], F16, tag="sel16")
            nc.sync.dma_start(sel16_t[:], sel16_d)

            # Fused-contraction operand tiles; aug rows replicated x8 by DMA
            lhs_all = consts.tile([KF, NH * S], F16, tag="lhs_all", name="lhs_all")
            rhs_all = consts.tile([KF, NH * S], F16, tag="rhs_all", name="rhs_all")
            nc.scalar.dma_start(lhs_all[RU:RU + 32, :], blk_ap(dlA_d, 32, S, NH, rep=True))
            nc.scalar.dma_start(lhs_all[RP1:RP1 + 4, :], blk_ap(dlB_d, 4, S, NH, rep=True))
            nc.sync.dma_start(rhs_all[RP0:RP0 + 20, :], blk_ap(drA_d, 20, S, NH, rep=True))
            nc.sync.dma_start(rhs_all[RS:RS + R1, :], blk_ap(drB_d, R1, S, NH, rep=True))
            nc.sync.dma_start(rhs_all[RP1:RP1 + 4, :], blk_ap(drC_d, 4, S, NH, rep=True))
            # Taylor rows 113(lhs)/114(rhs) are all-ones (host replicated)
            nc.scalar.dma_start(lhs_all[RT + 2:RT + 3, :],
                                blk_ap(ones_d, 1, S, NH, rep=True))
            nc.scalar.dma_start(rhs_all[RT + 1:RT + 2, :],
                                blk_ap(ones_d, 1, S, NH, rep=True))
            if flags["use_mask"]:
                nc.sync.dma_start(
                    lhs_all[KA:KA + 1, :],
                    bass.AP(tensor=m8_d.tensor, offset=0, ap=[[0, 1], [0, NH], [1, S]]))
                nc.scalar.dma_start(rhs_all[KA:KA + 1, :],
                                    blk_ap(ones_d, 1, S, NH, rep=True))

            wv16 = consts.tile([128, NT * S], F16, tag="wv16")
            nc.scalar.dma_start(wv16[:], blk_ap(wv_d, 128, S, NT))
            cret16 = consts.tile([128, NT * NF], F16, tag="cret16")
            nc.sync.dma_start(cret16[:], blk_ap(cret_d, 128, NF, NT))
            cimt16 = consts.tile([128, NT * NF], F16, tag="cimt16")
            nc.sync.dma_start(cimt16[:], blk_ap(cimt_d, 128, NF, NT))
            irA16 = consts.tile([128, 2 * S], F16, tag="irA16")
            nc.scalar.dma_start(irA16[:], blk_ap(irA_d, 128, S, 2))
            irB16 = consts.tile([128, 2 * S], F16, tag="irB16")
            nc.scalar.dma_start(irB16[:], blk_ap(irB_d, 128, S, 2))
            wrt16 = consts.tile([128, 2 * S], F16, tag="wrt16")
            nc.sync.dma_start(wrt16[:], blk_ap(wrt_d, 128, S, 2))
            wit16 = consts.tile([128, 2 * S], F16, tag="wit16")
            nc.sync.dma_start(wit16[:], blk_ap(wit_d, 128, S, 2))

            bias_cols = {}
            for nm, dd in (("bq", flags["use_bq"] and bq_d),
                           ("bk", flags["use_bk"] and bk_d)):
                if dd:
                    t = consts.tile([128, NT], F32, tag=nm)
                    nc.sync.dma_start(t[:], bass.AP(tensor=dd.tensor, offset=0,
                                                    ap=[[1, 128], [128, NT]]))
                    bias_cols[nm] = t
            if flags["use_bv"]:
                bv_row = consts.tile([1, H], F32, tag="bv")
                nc.sync.dma_start(bv_row[:], bass.AP(tensor=bv_d.tensor, offset=0,
                                                     ap=[[0, 1], [1, H]]))
            ln_bc = {}
            for nm, d_ap in ln_bcast.items():
                t = consts.tile([128, H], F32, tag=nm + "b")
                nc.gpsimd.dma_start(t[:], bass.AP(tensor=d_ap.tensor, offset=0,
                                                  ap=[[0, 128], [1, H]]))
                ln_bc[nm] = t

            # ---- small constants ----
            i16 = consts.tile([128, 128], F16, tag="i16")
            make_identity(nc, i16[:])
            i32 = consts.tile([8, 8], F32, tag="i32")
            make_identity(nc, i32[:])
            i32r = consts.tile([8, 8], F32, tag="i32r")
            nc.vector.tensor_copy(i32r[:].bitcast(R), i32[:])
            onescol0 = consts.tile([1, 128], F32, tag="onescol0")
            nc.vector.memset(onescol0[:], 1.0)
            onescol = consts.tile([1, 128], F32, tag="onescol")
            nc.vector.tensor_copy(onescol[:].bitcast(R), onescol0[:])
            ones_f = consts.tile([128, NH], F32, tag="ones_f")
            nc.vector.memset(ones_f[:], 1.0)
            _ccols = {}

            def constcol(val):
                if val not in _ccols:
                    t = consts.tile([128, 1], F32, tag=f"cc{len(_ccols)}")
                    nc.vector.memset(t[:], val)
                    _ccols[val] = t
                return _ccols[val]

            # ---- prologue: X^T ----
            xt16 = []
            diagp = ctx.enter_context(tc.tile_pool(name="diagp", bufs=6))
            with ExitStack() as pctx:
                pA = pctx.enter_context(tc.tile_pool(name="pA", bufs=2, space="PSUM"))
                pB = pctx.enter_context(tc.tile_pool(name="pB", bufs=4, space="PSUM"))
                for ht in range(NT):
                    tp = pA.tile([128, S], F16, tag="tp")
                    for st in range(NT):
                        nc.tensor.transpose(
                            tp[:, st * 128:(st + 1) * 128],
                            x16[:, st * S + ht * 128: st * S + (ht + 1) * 128],
                            i16[:])
                    t = consts.tile([128, S], F16, tag=f"xt{ht}", name=f"xt{ht}")
                    nc.vector.tensor_copy(t[:], tp[:])
                    if dbg and ht == 0:
                        nc.sync.dma_start(dbg_d["d_xt0"], t[:])
                    xt16.append(t)

                # ---- rows: oq/dq (q side), ok/dk (k side); separate base-0
                # PSUM groups per 8-row output ----
                def rows8(wblk, colbase, name, dt16, bias, eng):
                    psf = pB.tile([128, S], F32, tag="pj")
                    ps = psf[0:8, :]
                    for ht in range(NT):
                        nc.tensor.matmul(ps, wblk[:, ht * 16 + colbase:ht * 16 + colbase + 8],
                                         xt16[ht][:], start=(ht == 0),
                                         stop=(ht == NT - 1))
                    t = consts.tile([8, S], F16 if dt16 else F32, tag=name)
                    tout = t[:] if dt16 else t[:].bitcast(R)
                    if bias != 0.0:
                        nc.scalar.activation(tout, ps, AF.Identity,
                                             bias=constcol(float(bias))[0:8, 0:1],
                                             scale=1.0)
                    else:
                        eng.tensor_copy(tout, ps)
                    return t

                rows_oq = rows8(wblkq16[:], 0, "r_oq", False, c["b_order"], nc.vector)
                rows16_dq = rows8(wblkq16[:], 8, "r_dq", True, 0.0, nc.vector)
                rows_ok = rows8(wblkk16[:], 0, "r_ok", False, 0.0, nc.vector)
                rows16_dk = rows8(wblkk16[:], 8, "r_dk", True, c["b_dist"], nc.vector)

                # ok/oq columns [128,8] per block (diag-build scalar ptrs)
                okc, oqc = [], []
                for kt in range(NT):
                    ps = pB.tile([128, S], F32, tag="pj")
                    nc.tensor.matmul(ps[:, 0:8],
                                     rows_ok[:, kt * 128:(kt + 1) * 128].bitcast(R),
                                     i32r[:].bitcast(R), start=True, stop=False)
                    nc.tensor.matmul(ps[:, 8:16],
                                     rows_oq[:, kt * 128:(kt + 1) * 128].bitcast(R),
                                     i32r[:].bitcast(R), start=False, stop=True)
                    t = consts.tile([128, 16], F32, tag=f"okc{kt}")
                    nc.vector.tensor_copy(t[:], ps[:, 0:16])
                    okc.append(t)
                    oqc.append(t)

                # Taylor staging rows (all heads): w'=-ok^2/8, u'=-oq^2/8,
                # -oq/4, ok (fp16), then flat SBUF->SBUF DMAs scatter them
                # into aug rows 112..114 of each head block.
                ok16s = consts.tile([8, S], F16, tag="ok16s")
                nc.scalar.copy(ok16s[:], rows_ok[:])
                sqk = consts.tile([8, S], F16, tag="sqk")
                nc.scalar.square(sqk[:], rows_ok[:])
                w16s = consts.tile([8, S], F16, tag="w16s")
                nc.scalar.activation(w16s[:], sqk[:], AF.Identity, scale=-0.125)
                tq16s = consts.tile([8, S], F16, tag="tq16s")
                nc.scalar.activation(tq16s[:], rows_oq[:], AF.Identity, scale=-0.25)
                squ = consts.tile([8, S], F16, tag="squ")
                nc.scalar.square(squ[:], rows_oq[:])
                u16s = consts.tile([8, S], F16, tag="u16s")
                nc.scalar.activation(u16s[:], squ[:], AF.Identity, scale=-0.125)

                def flat_row(dst_row, srct):
                    nc.sync.dma_start(dst_row, srct[:])

                flat_row(lhs_all[RT:RT + 1, :], ok16s)       # ok  | -oq/4
                flat_row(rhs_all[RT:RT + 1, :], tq16s)
                flat_row(lhs_all[RT + 1:RT + 2, :], w16s)    # w'  | ones
                flat_row(rhs_all[RT + 2:RT + 3, :], u16s)    # ones| u' 

                # ---- per-head scaled SVD basis rows ----
                for h in range(NH):
                    hsl = slice(h * S, (h + 1) * S)
                    dkb = pB.tile([128, S], F32, tag="pj")
                    nc.tensor.matmul(dkb[0:R1, :], sel16_t[:, h * R1:(h + 1) * R1],
                                     rows16_dk[:], start=True, stop=True)
                    nc.vector.scalar_tensor_tensor(
                        lhs_all[RS:RS + R1, hsl],
                        lhs_all[RU:RU + R1, hsl], twoc, dkb[0:R1, :],
                        op0=ALU.mult, op1=ALU.mult)
                    dqb = pB.tile([128, S], F32, tag="pj")
                    nc.tensor.matmul(dqb[0:R1, :], sel16_t[:, h * R1:(h + 1) * R1],
                                     rows16_dq[:], start=True, stop=True)
                    nc.vector.scalar_tensor_tensor(
                        rhs_all[RU:RU + R1, hsl],
                        rhs_all[RS:RS + R1, hsl], twoc, dqb[0:R1, :],
                        op0=ALU.mult, op1=ALU.mult)

                # ---- projections: q/k head rows into rhs_all/lhs_all ----
                for ot in range(NT):
                    psq = pB.tile([128, S], F32, tag="pj")
                    for ht in range(NT):
                        nc.tensor.matmul(psq[:], wq16[:, ht * S + ot * 128:ht * S + (ot + 1) * 128],
                                         xt16[ht][:], start=(ht == 0), stop=(ht == NT - 1))
                    psk = pB.tile([128, S], F32, tag="pj")
                    for ht in range(NT):
                        nc.tensor.matmul(psk[:], wk16[:, ht * S + ot * 128:ht * S + (ot + 1) * 128],
                                         xt16[ht][:], start=(ht == 0), stop=(ht == NT - 1))
                    for po, h in ((0, 2 * ot), (64, 2 * ot + 1)):
                        hsl = slice(h * S, (h + 1) * S)
                        if flags["use_bq"]:
                            nc.scalar.activation(rhs_all[0:KQ, hsl], psq[po:po + 64, :],
                                                 AF.Identity,
                                                 bias=bias_cols["bq"][po:po + 64, ot:ot + 1],
                                                 scale=1.0)
                        else:
                            nc.scalar.activation(rhs_all[0:KQ, hsl], psq[po:po + 64, :],
                                                 AF.Identity)
                        if flags["use_bk"]:
                            nc.scalar.activation(lhs_all[0:KQ, hsl], psk[po:po + 64, :],
                                                 AF.Identity,
                                                 bias=bias_cols["bk"][po:po + 64, ot:ot + 1],
                                                 scale=1.0)
                        else:
                            nc.vector.tensor_copy(lhs_all[0:KQ, hsl], psk[po:po + 64, :])

                # ---- V projection -> vaug (value rows + ones column) ----
                vaug = []
                for st in range(NT):
                    ps = pB.tile([128, S], F32, tag="pj")
                    for ht in range(NT):
                        nc.tensor.matmul(ps[:], xt16[ht][:, st * 128:(st + 1) * 128],
                                         wv16[:, ht * S:(ht + 1) * S],
                                         start=(ht == 0),
                                         stop=(ht == NT - 1 and not flags["use_bv"]))
                    if flags["use_bv"]:
                        nc.tensor.matmul(ps[:], onescol[:].bitcast(R),
                                         bv_row[:].bitcast(R), start=False, stop=True)
                    t = consts.tile([128, NH * 65], F32, tag=f"vaug{st}", name=f"vaug{st}")
                    tap = t[:]
                    ones_cols = bass.AP(tensor=tap.tensor, offset=tap.offset + D,
                                        ap=[list(tap.ap[0]), [65, NH], [1, 1]])
                    nc.vector.tensor_copy(ones_cols.bitcast(R), ones_f[:])
                    dst = bass.AP(tensor=tap.tensor, offset=tap.offset,
                                  ap=[list(tap.ap[0]), [65, NH], [1, D]])
                    nc.vector.tensor_copy(dst.bitcast(R), ps[:])
                    vaug.append(t)
            # ---- head loop ----
            ctxt16 = [consts.tile([128, S], F16, tag=f"ctxt{ht}", name=f"ctxt{ht}")
                      for ht in range(NT)]
            if dbg:
                nc.sync.dma_start(dbg_d["d_lhs"], lhs_all[0:KF, :])
                nc.sync.dma_start(dbg_d["d_rhs"], rhs_all[0:KF, :])
                nc.sync.dma_start(dbg_d["d_rows_oq"], rows_oq[:])
                nc.sync.dma_start(dbg_d["d_okc0"], okc[0][:])
            rfp = ctx.enter_context(tc.tile_pool(name="rfp", bufs=1, space="PSUM"))
            rt0_ps = rfp.tile([128, S], F32, tag="rt0")
            it0_ps = rfp.tile([128, S], F32, tag="it0")
            with ExitStack() as lctx:
                scp = lctx.enter_context(
                    tc.tile_pool(name="scp", bufs=4, space="PSUM"))
                ctxp = lctx.enter_context(
                    tc.tile_pool(name="ctxp", bufs=1, space="PSUM"))
                rbpp = lctx.enter_context(
                    tc.tile_pool(name="rbpp", bufs=1, space="PSUM"))
                for h in range(NH):
                    hb = h * S
                    et = etp.tile([128, NT * S], R, tag="et")
                    cps = ctxp.tile([65, S], F32, tag="cps")

                    def ctx_mm(kt):
                        nc.tensor.matmul(cps[:], vaug[kt][:, h * 65:(h + 1) * 65].bitcast(R),
                                         et[:, kt * S:(kt + 1) * S],
                                         start=(kt == 0), stop=(kt == NT - 1))

                    doq = []
                    for qt in range(NT):
                        dt_ = diagp.tile([128, 128], F16, tag="doq")
                        nc.gpsimd.tensor_scalar_mul(dt_[:], i16[:],
                                                    oqc[qt][:, 8 + h:9 + h])
                        doq.append(dt_)
                    for kt in range(NT):
                        o = scp.tile([128, S], F32, tag="sc")
                        nc.tensor.matmul(o[:], lhs_all[0:KF, hb + kt * 128:hb + (kt + 1) * 128],
                                         rhs_all[0:KF, hb:hb + S], start=True, stop=False)
                        dok = diagp.tile([128, 128], F16, tag="dok")
                        nc.gpsimd.tensor_scalar_mul(dok[:], i16[:],
                                                    okc[kt][:, h:h + 1])
                        nc.tensor.matmul(o[:], dok[:], lm_t[:, kt * S:(kt + 1) * S],
                                         start=False, stop=False)
                        for qt in range(NT):
                            nc.tensor.matmul(
                                o[:, qt * 128:(qt + 1) * 128],
                                lmt_t[:, qt * S + kt * 128:qt * S + (kt + 1) * 128],
                                doq[qt][:], start=False, stop=(qt == NT - 1))
                        nc.scalar.activation(et[:, kt * S:(kt + 1) * S], o[:],
                                             AF.Exp, scale=0.125)
                        if kt >= 2:
                            ctx_mm(kt - 2)
                    ctx_mm(2)
                    ctx_mm(3)
                    # normalization: reciprocal of the denom row, broadcast via
                    # PE, fp16 copy + multiply on Pool
                    if dbg and h == 0:
                        etsb = work.tile([128, NT * S], F32, tag="dbget")
                        nc.vector.tensor_copy(etsb[:], et[:])
                        nc.sync.dma_start(dbg_d["d_et0"], etsb[:])
                        cpsb = work.tile([65, S], F32, tag="dbgcps")
                        nc.vector.tensor_copy(cpsb[:], cps[:])
                        nc.sync.dma_start(dbg_d["d_cps0"], cpsb[:])
                    rc = small.tile([1, S], mybir.dt.float32r, tag="rc")
                    with nc.allow_low_precision(reason="softmax denom reciprocal to f32r"):
                        nc.vector.reciprocal(rc[:], cps[64:65, :])
                    rbp = rbpp.tile([64, S], F32, tag="rbp")
                    nc.tensor.matmul(rbp[:], onescol[0:1, 0:64].bitcast(R), rc[:],
                                     start=True, stop=True)
                    rbs = work.tile([64, S], F16, tag="rbs")
                    nc.vector.tensor_copy(rbs[:], rbp[:])
                    po = (h % 2) * 64
                    nc.vector.tensor_tensor(ctxt16[h // 2][po:po + 64, :],
                                            cps[0:64, :], rbs[:], ALU.mult)
                    if h % 2 == 1:
                        ht = h // 2
                        nc.tensor.matmul(rt0_ps[:], cret16[:, ht * NF:ht * NF + 128],
                                         ctxt16[ht][:], start=(ht == 0), stop=(ht == NT - 1))
                        nc.tensor.matmul(it0_ps[:], cimt16[:, ht * NF:ht * NF + 128],
                                         ctxt16[ht][:], start=(ht == 0), stop=(ht == NT - 1))

            if dbg:
                nc.sync.dma_start(dbg_d["d_ctxt0"], ctxt16[0][:])
            # ---- FFT filter + residual + layernorms (tail) ----
            with ExitStack() as fctx:
                fftp = fctx.enter_context(
                    tc.tile_pool(name="fftp", bufs=1, space="PSUM"))
                miscp = fctx.enter_context(
                    tc.tile_pool(name="miscp", bufs=1, space="PSUM"))
                tpool = fctx.enter_context(tc.tile_pool(name="tpool", bufs=4))
                pr16, pi16 = [], []
                for ft in range(2):
                    if ft == 0:
                        rt_ps, it_ps = rt0_ps, it0_ps
                    else:
                        rt_ps = fftp.tile([128, S], F32, tag=f"rt{ft}")
                        it_ps = fftp.tile([128, S], F32, tag=f"it{ft}")
                        for ht in range(NT):
                            nc.tensor.matmul(rt_ps[:], cret16[:, ht * NF + ft * 128:ht * NF + (ft + 1) * 128],
                                             ctxt16[ht][:], start=(ht == 0), stop=(ht == NT - 1))
                        for ht in range(NT):
                            nc.tensor.matmul(it_ps[:], cimt16[:, ht * NF + ft * 128:ht * NF + (ft + 1) * 128],
                                             ctxt16[ht][:], start=(ht == 0), stop=(ht == NT - 1))
                    rts = work.tile([128, S], F16, tag="rts")
                    nc.vector.tensor_copy(rts[:], rt_ps[:])
                    its = work.tile([128, S], F16, tag="its")
                    nc.vector.tensor_copy(its[:], it_ps[:])
                    wrs = wrt16[:, ft * S:(ft + 1) * S]
                    wis = wit16[:, ft * S:(ft + 1) * S]
                    t1 = work.tile([128, S], F16, tag="f1")
                    t2 = work.tile([128, S], F16, tag="f2")
                    nc.vector.tensor_tensor(t1[:], rts[:], wrs, ALU.mult)
                    nc.vector.tensor_tensor(t2[:], its[:], wis, ALU.mult)
                    pr = consts.tile([128, S], F16, tag=f"pr{ft}", name=f"pr{ft}")
                    nc.vector.tensor_tensor(pr[:], t1[:], t2[:], ALU.subtract)
                    pr16.append(pr)
                    nc.vector.tensor_tensor(t1[:], rts[:], wis, ALU.mult)
                    nc.vector.tensor_tensor(t2[:], its[:], wrs, ALU.mult)
                    pi = consts.tile([128, S], F16, tag=f"pi{ft}", name=f"pi{ft}")
                    nc.vector.tensor_tensor(pi[:], t1[:], t2[:], ALU.add)
                    pi16.append(pi)

                def ln_stats(src, tagn):
                    st6 = small.tile([128, 6], F32, tag="st6" + tagn)
                    nc.vector.bn_stats(st6[:], src)
                    mv = small.tile([128, 2], F32, tag="mv" + tagn)
                    nc.vector.bn_aggr(mv[:], st6[:])
                    lnv = small.tile([128, 1], F32, tag="lnv" + tagn)
                    nc.scalar.activation(lnv[:], mv[:, 1:2], AF.Ln,
                                         bias=constcol(1e-12)[:, 0:1], scale=1.0)
                    rs = small.tile([128, 1], F32, tag="rs" + tagn)
                    nc.scalar.activation(rs[:], lnv[:], AF.Exp, scale=-0.5)
                    nb = small.tile([128, 1], F32, tag="nb" + tagn)
                    nc.vector.scalar_tensor_tensor(
                        nb[:], mv[:, 0:1], -1.0, rs[:],
                        op0=ALU.mult, op1=ALU.mult)
                    return rs, nb

                def ln_apply(dst, src, rs, nb, wname, bname):
                    nc.scalar.activation(dst, src, AF.Identity,
                                         bias=nb[:, 0:1], scale=rs[:, 0:1])
                    if flags["use_" + wname]:
                        nc.vector.tensor_mul(dst, dst, ln_bc[wname][:])
                    if flags["use_" + bname]:
                        nc.vector.tensor_add(dst, dst, ln_bc[bname][:])

                yps, rss1, nbs1 = [], [], []
                for st in range(NT):
                    ssl = slice(st * 128, (st + 1) * 128)
                    yp = miscp.tile([128, S], F32, tag=f"yp{st}")
                    for ft in range(2):
                        nc.tensor.matmul(yp[:], pr16[ft][:, ssl],
                                         irA16[:, ft * S:(ft + 1) * S],
                                         start=(ft == 0), stop=False)
                        nc.tensor.matmul(yp[:], pi16[ft][:, ssl],
                                         irB16[:, ft * S:(ft + 1) * S],
                                         start=False, stop=False)
                    for ht in range(NT):
                        nc.tensor.matmul(yp[:, ht * 128:(ht + 1) * 128],
                                         ctxt16[ht][:, ssl], i16[:],
                                         start=False, stop=(ht == NT - 1))
                    rs, nb = ln_stats(yp[:], f"a{st}")
                    yps.append(yp)
                    rss1.append(rs)
                    nbs1.append(nb)
                r2s, rss2, nbs2 = [], [], []
                for st in range(NT):
                    hid = work.tile([128, S], F32, tag="hid")
                    ln_apply(hid[:], yps[st][:], rss1[st], nbs1[st], "lnfw", "lnfb")
                    r2t = tpool.tile([128, S], F32, tag="r2")
                    nc.vector.tensor_add(r2t[:], hid[:], x16[:, st * S:(st + 1) * S])
                    r2s.append(r2t)
                for st in range(NT):
                    rs, nb = ln_stats(r2s[st][:], f"b{st}")
                    rss2.append(rs)
                    nbs2.append(nb)
                for st in range(NT):
                    ssl = slice(st * 128, (st + 1) * 128)
                    osb = work.tile([128, S], F32, tag="osb")
                    ln_apply(osb[:], r2s[st][:], rss2[st], nbs2[st], "lnw", "lnb")
                    nc.sync.dma_start(out_d[ssl, :], osb[:])

    nsplit = _split_excess_waits(nc)
    if nsplit:
        print(f"[kernel2] split {nsplit} excess sync waits onto NOPs")
    return nc


_CACHE = {}
LAST_EXEC_NS = None
LAST_RESULTS = None


def _flags_cvals(inputs):
    import os
    flags = {
        "use_mask": bool(np.any(inputs["attention_mask"] != 0)),
        "use_bq": bool(np.any(inputs["bq"] != 0)),
        "use_bk": bool(np.any(inputs["bk"] != 0)),
        "use_bv": bool(np.any(inputs["bv"] != 0)),
        "use_lnfw": not bool(np.all(inputs["ln_f_w"] == 1.0)),
        "use_lnfb": bool(np.any(inputs["ln_f_b"] != 0)),
        "use_lnw": not bool(np.all(inputs["ln_w"] == 1.0)),
        "use_lnb": bool(np.any(inputs["ln_b"] != 0)),
        "use_softplus": os.environ.get("KERNEL_SOFTPLUS", "") == "1",
    }
    cvals = {
        "c": float(inputs["scalar"][0]) ** 2 / 2.0,
        "b_order": float(inputs["b_order"][0]),
        "b_dist": float(inputs["b_dist"][0]),
    }
    return flags, cvals


def _shared_inputs(inputs, flags, cvals):
    hc = _host_constants()
    c = cvals["c"]
    Wq = inputs["Wq"].astype(np.float64)
    Wk = inputs["Wk"].astype(np.float64)
    wo, wd = inputs["W_order"].astype(np.float64), inputs["W_dist"].astype(np.float64)
    wblkq = np.zeros((H, 16), np.float64)
    wblkk = np.zeros((H, 16), np.float64)
    for h in range(NH):
        hs = slice(h * D, (h + 1) * D)
        wblkq[:, h] = Wq[:, hs] @ wo[:D, 0]
        wblkq[:, 8 + h] = Wq[:, hs] @ wd[:D, 0]
        wblkk[:, h] = Wk[:, hs] @ wo[D:, 0]
        wblkk[:, 8 + h] = Wk[:, hs] @ wd[D:, 0]
    cw = inputs["complex_weight"].astype(np.float32)
    shared = {
        "wq": inputs["Wq"].astype(np.float16),
        "wk": inputs["Wk"].astype(np.float16),
        "wv": inputs["Wv"].astype(np.float16),
        "wblkq": wblkq.astype(np.float16),
        "wblkk": wblkk.astype(np.float16),
        "lm": hc["Lm"], "lmt": hc["LmT"],
        "ones1": np.ones((1, S), np.float16),
        "dlA": np.vstack([hc["U1"].T, hc["P2"].T[0:20]]).astype(np.float16),
        "dlB": hc["P2"].T[20:24].astype(np.float16),
        "drA": (-c * hc["S2c"].T[0:20]).astype(np.float16),
        "drB": hc["V1"].T.astype(np.float16),
        "drC": (-c * hc["S2c"].T[20:24]).astype(np.float16),
        "cret": hc["cret"], "cimt": hc["cimt"],
        "irA": hc["irA"], "irB": hc["irB"],
        "wrt": np.ascontiguousarray(cw[0, :, :NF, 0].T).astype(np.float16),
        "wit": np.ascontiguousarray(cw[0, :, :NF, 1].T).astype(np.float16),
        "sel16": hc["sel16"],
    }
    if flags["use_bq"]:
        shared["bq"] = inputs["bq"].astype(np.float32)
    if flags["use_bk"]:
        shared["bk"] = inputs["bk"].astype(np.float32)
    if flags["use_bv"]:
        shared["bv"] = inputs["bv"].astype(np.float32)
    for nm, src in (("lnfw", "ln_f_w"), ("lnfb", "ln_f_b"),
                    ("lnw", "ln_w"), ("lnb", "ln_b")):
        if flags["use_" + nm]:
            shared[nm] = inputs[src].astype(np.float32)
    return shared


def kernel(**inputs):
    inputs = {k: np.asarray(v) for k, v in inputs.items()}
    x_all = inputs["input_tensor"].astype(np.float32)
    mask = inputs["attention_mask"].astype(np.float32)
    flags, cvals = _flags_cvals(inputs)

    key = (tuple(sorted(flags.items())), tuple(sorted(cvals.items())))
    if key not in _CACHE:
        _CACHE[key] = _build_program(cvals, flags)
    nc = _CACHE[key]

    shared = _shared_inputs(inputs, flags, cvals)
    in_maps = []
    for b in range(B):
        m = dict(shared)
        m["x"] = np.ascontiguousarray(x_all[b]).astype(np.float16)
        if flags["use_mask"]:
            m["m8"] = np.ascontiguousarray(8.0 * mask[b, 0, 0, :]).astype(np.float16)
        in_maps.append(m)

    import os
    trace = os.environ.get("KERNEL_TRACE", "") == "1"
    res = run_bass_kernel_spmd(nc, in_maps, core_ids=list(range(B)), trace=trace)
    global LAST_EXEC_NS, LAST_RESULTS
    LAST_RESULTS = res
    if res.exec_time_ns is not None:
        LAST_EXEC_NS = res.exec_time_ns
    out = np.stack([res.results[b]["out"] for b in range(B)]).astype(np.float32)
    return out


if __name__ == "__main__":
    print("kernel2 module ok")
